# revision 17
# baseline (speedup 1.0000x reference)
"""DropEdge GraphSAGE (eval mode) on 8 Trainium2 NeuronCores.

Strategy (graph/data parallel, per sharding hint):
- Nodes padded 50000->50176 = 8 cores x 6272 (49 blocks of 128).
- Edges sharded by destination core; per core grouped by (dst block, src
  parity), sorted by src, padded to 128-edge tiles (uniform tile counts
  across cores so the SPMD program is identical).
- Layer 0 (x @ W_in + LN): computed fully on every core (replicated) to
  avoid an AllGather; own-shard h/hn also kept in SBUF.
- Aggregation per layer: indirect DMA gather of *row pairs* (1024B
  descriptors, idx = src>>1 fits int16) from the replicated hn table in
  DRAM; one-hot selection matrices S (built on DVE from iota vs dst ids,
  with 1/deg folded in) turn segment-sum into PE matmuls accumulating in
  PSUM per dst block, feature-major: aggT = G^T S.
- Dense part: conv = aggT^T Wl + hnT^T Wr in PSUM; residual+relu; LN via
  bn_stats/bn_aggr; AllGather of the new hn shard between layers.
- Final layer: h3^T W_out -> per-core [6272,4]; host concatenates + trims.
"""
import sys
sys.path.insert(0, "/opt/trn_rl_repo")
import numpy as np

N_NODES = 50000
N_EDGES = 800000
D_IN = 16
HID = 128
D_OUT = 4
N_LAYERS = 3
LN_EPS = 1e-5

P = 128
NCORES = 8
NP = 50176            # padded nodes
SH = NP // NCORES     # 6272 per core
NB = SH // P          # 49 blocks per core
NT_FULL = NP // P     # 392 node tiles
WIN = 8               # tiles per gather window (<=1024 idxs)

_CACHE = {}


def _host_prep(edge_index):
    src = np.asarray(edge_index[0], dtype=np.int64)
    dst = np.asarray(edge_index[1], dtype=np.int64)
    deg = np.bincount(dst, minlength=NP).astype(np.float32)
    inv_deg = 1.0 / np.maximum(deg, 1.0)

    core = dst // SH
    block = (dst % SH) // P
    # sort edges by (core, block, src)
    order = np.lexsort((src, block, core))
    s_src, s_dst, s_core, s_blk = src[order], dst[order], core[order], block[order]
    key = s_core * NB + s_blk
    cnt = np.bincount(key, minlength=NCORES * NB).reshape(NCORES, NB)
    # uniform tiles per block = max over cores (>=1 so PSUM is initialized)
    tiles_b = np.maximum(np.ceil(cnt.max(axis=0) / P).astype(np.int64), 1)  # [NB]
    TT = int(tiles_b.sum())

    tile_block = np.zeros(TT, np.int64)
    seg_off = np.zeros(NB, np.int64)
    t = 0
    for b in range(NB):
        seg_off[b] = t
        n = int(tiles_b[b])
        tile_block[t:t + n] = b
        t += n
    assert t == TT

    # Table rows are PERMUTED: each core's blocks 0..24 (the "A" half,
    # all-gathered mid-layer) land in high rows [24576, 50176); blocks
    # 25..48 ("B", all-gathered at layer end) land in [0, 24576). The
    # signed-idx gather AP starts at 32768 (inside A), so Tile auto-deps
    # gathers on the A collective; B gets explicit deps per window.
    nc_ = np.arange(NP) // SH
    nr_ = np.arange(NP) % SH
    nblk = nr_ // P
    rowmap = np.where(
        nblk < 25,
        24576 + nc_ * 3200 + nr_,
        nc_ * 3072 + (nr_ - 25 * P))
    PAD_IDX = 50175 - 32768  # an A-region row; harmless (S column zero)
    pidx = np.full((NCORES, TT * P), PAD_IDX, np.int64)
    dcol = np.full((NCORES, TT * P), -1.0, np.float32)
    vcol = np.zeros((NCORES, TT * P), np.float32)
    starts = np.concatenate([[0], np.cumsum(cnt.reshape(-1))])
    for c in range(NCORES):
        for b in range(NB):
            k = c * NB + b
            lo, hi = starts[k], starts[k + 1]
            n = hi - lo
            if n == 0:
                continue
            off = seg_off[b] * P
            rows = rowmap[s_src[lo:hi]]
            o2 = np.argsort(-rows, kind="stable")  # A-rows (high) first
            pidx[c, off:off + n] = rows[o2] - 32768
            dcol[c, off:off + n] = ((s_dst[lo:hi] % SH) % P)[o2]
            vcol[c, off:off + n] = inv_deg[s_dst[lo:hi]][o2]

    # Q7 drops trailing-negative idxs per call: the LAST idx of every
    # 1024-idx gather window must be >= 0. Swap a non-negative idx (high
    # src or pad) from the same segment (same dst block -> semantics
    # unchanged) into each bad window's last slot.
    nwin = (TT + WIN - 1) // WIN
    win_last = set((min((w + 1) * WIN, TT) * P) - 1 for w in range(nwin))
    seg_lo = seg_off * P                      # edge offset of each segment
    seg_hi = (seg_off + tiles_b) * P
    for c in range(NCORES):
        for w in range(nwin):
            e1 = min((w + 1) * WIN, TT) * P
            j = e1 - 1
            if pidx[c, j] >= 0:
                continue
            b = int(tile_block[(e1 - 1) // P])
            cand = seg_lo[b] + np.nonzero(pidx[c, seg_lo[b]:seg_hi[b]] >= 0)[0]
            cand = [int(q) for q in cand if int(q) not in win_last]
            if not cand:
                raise ValueError("segment with no high-src edge or pad; "
                                 "unsupported input distribution")
            q = cand[-1]
            for arr in (pidx, dcol, vcol):
                arr[c, q], arr[c, j] = arr[c, j], arr[c, q]

    # classify windows AFTER swaps: window needs the B collective iff any
    # core's window touches a row < 24576 (same flag on all cores - SPMD)
    win_b = np.zeros(nwin, bool)
    for w in range(nwin):
        e0, e1 = w * WIN * P, min((w + 1) * WIN, TT) * P
        win_b[w] = bool((pidx[:, e0:e1] + 32768 < 24576).any())

    idx16 = np.tile(
        pidx.astype(np.int16).reshape(NCORES, TT * P // 16, 16).transpose(0, 2, 1),
        (1, 8, 1))  # [NCORES, 128, TT*8]
    dcol = dcol.reshape(NCORES, TT, P).transpose(0, 2, 1)
    vcol = vcol.reshape(NCORES, TT, P).transpose(0, 2, 1)

    return dict(idx16=idx16, dcol=np.ascontiguousarray(dcol),
                vcol=np.ascontiguousarray(vcol),
                tiles_b=tiles_b, tile_block=tile_block, TT=TT, win_b=win_b)


def _build_program(meta, use_bin, use_bl, use_g, use_bout):
    import concourse.bacc as bacc
    import concourse.mybir as mybir
    import concourse.tile as tile
    from concourse.tile_rust import add_dep_helper
    from concourse.alu_op_type import AluOpType as ALU

    FP32 = mybir.dt.float32
    I16 = mybir.dt.int16
    AF = mybir.ActivationFunctionType

    TT = meta["TT"]
    tile_block = meta["tile_block"]
    win_b = meta["win_b"]
    K0 = D_IN + (1 if use_bin else 0)

    nc = bacc.Bacc("TRN2", target_bir_lowering=False, debug=False,
                   num_devices=NCORES)

    # ---- I/O ----
    xTo = nc.dram_tensor("xTo", [K0, SH], FP32, kind="ExternalInput")
    Wa = nc.dram_tensor("Wa", [K0, HID], FP32, kind="ExternalInput")
    Wl = nc.dram_tensor("Wl", [N_LAYERS, HID, HID], FP32, kind="ExternalInput")
    Wr = nc.dram_tensor("Wr", [N_LAYERS, HID, HID], FP32, kind="ExternalInput")
    Wout = nc.dram_tensor("Wout", [HID, D_OUT], FP32, kind="ExternalInput")
    iota_d = nc.dram_tensor("iota", [P, P], FP32, kind="ExternalInput")
    ident_d = nc.dram_tensor("ident", [P, P], FP32, kind="ExternalInput")
    idx_d = nc.dram_tensor("idx16", [P, TT * 8], I16, kind="ExternalInput")
    dcol_d = nc.dram_tensor("dcol", [P, TT], FP32, kind="ExternalInput")
    vcol_d = nc.dram_tensor("vcol", [P, TT], FP32, kind="ExternalInput")
    if use_bl:
        blb_d = nc.dram_tensor("blb", [N_LAYERS, P, HID], FP32, kind="ExternalInput")
    if use_g:
        gb_d = nc.dram_tensor("gb", [N_LAYERS, P, HID], FP32, kind="ExternalInput")
        bb_d = nc.dram_tensor("bb", [N_LAYERS, P, HID], FP32, kind="ExternalInput")
    if use_bout:
        bob_d = nc.dram_tensor("bob", [P, D_OUT], FP32, kind="ExternalInput")
    out_d = nc.dram_tensor("out", [SH, D_OUT], FP32, kind="ExternalOutput")

    # ---- internal DRAM ----
    hn0_d = nc.dram_tensor("hn0", [NP, HID], FP32, addr_space="Shared")
    ag_inA = nc.dram_tensor("ag_inA", [25 * P, HID], FP32)
    ag_inB = nc.dram_tensor("ag_inB", [24 * P, HID], FP32)
    ag1_d = nc.dram_tensor("ag1", [NP, HID], FP32, addr_space="Shared")
    ag2_d = nc.dram_tensor("ag2", [NP, HID], FP32, addr_space="Shared")

    with tile.TileContext(nc) as tc:
        with (
            tc.tile_pool(name="const", bufs=1) as cp,
            tc.tile_pool(name="resid", bufs=1) as rp,
            tc.tile_pool(name="work", bufs=4) as wp,
            tc.tile_pool(name="stat", bufs=4) as stp,
            tc.tile_pool(name="pagg", bufs=3, space="PSUM") as pagg,
            tc.tile_pool(name="pmisc", bufs=4, space="PSUM") as pmisc,
        ):
            # ---- constants into SBUF ----
            iota_t = cp.tile([P, P], FP32)
            nc.sync.dma_start(out=iota_t[:], in_=iota_d[:, :])
            ident_t = cp.tile([P, P], FP32)
            nc.sync.dma_start(out=ident_t[:], in_=ident_d[:, :])
            Wa_t = cp.tile([K0, HID], FP32)
            nc.sync.dma_start(out=Wa_t[:], in_=Wa[:, :])
            Wl_t = [cp.tile([HID, HID], FP32, tag=f"wl{i}", name=f"wl{i}") for i in range(3)]
            Wr_t = [cp.tile([HID, HID], FP32, tag=f"wr{i}", name=f"wr{i}") for i in range(3)]
            for i in range(3):
                nc.sync.dma_start(out=Wl_t[i][:], in_=Wl[i, :, :])
                nc.sync.dma_start(out=Wr_t[i][:], in_=Wr[i, :, :])
            Wout_t = cp.tile([HID, D_OUT], FP32)
            nc.sync.dma_start(out=Wout_t[:], in_=Wout[:, :])
            idx_t = cp.tile([P, TT * 8], I16)
            nc.sync.dma_start(out=idx_t[:], in_=idx_d[:, :])
            dcol_t = cp.tile([P, TT], FP32)
            nc.sync.dma_start(out=dcol_t[:], in_=dcol_d[:, :])
            vcol_t = cp.tile([P, TT], FP32)
            nc.sync.dma_start(out=vcol_t[:], in_=vcol_d[:, :])
            if use_bl:
                blb_t = [cp.tile([P, HID], FP32, tag=f"blb{i}", name=f"blb{i}") for i in range(3)]
                for i in range(3):
                    nc.sync.dma_start(out=blb_t[i][:], in_=blb_d[i, :, :])
            if use_g:
                gb_t = [cp.tile([P, HID], FP32, tag=f"gb{i}", name=f"gb{i}") for i in range(3)]
                bb_t = [cp.tile([P, HID], FP32, tag=f"bb{i}", name=f"bb{i}") for i in range(3)]
                for i in range(3):
                    nc.sync.dma_start(out=gb_t[i][:], in_=gb_d[i, :, :])
                    nc.sync.dma_start(out=bb_t[i][:], in_=bb_d[i, :, :])
            if use_bout:
                bob_t = cp.tile([P, D_OUT], FP32)
                nc.sync.dma_start(out=bob_t[:], in_=bob_d[:, :])

            eps_t = cp.tile([P, 1], FP32)
            nc.vector.memset(eps_t[:], LN_EPS)
            h_blk = [rp.tile([P, HID], FP32, tag=f"h{b}", name=f"h{b}") for b in range(NB)]
            hn_blk = [rp.tile([P, HID], FP32, tag=f"hn{b}", name=f"hn{b}") for b in range(NB)]

            def layer_norm_tile(src_ap, dst_ap, li):
                """dst = LN(src) (optionally *g+b). src may be PSUM."""
                st6 = stp.tile([P, 6], FP32, tag="st6")
                nc.vector.bn_stats(st6[:], src_ap)
                mv = stp.tile([P, 2], FP32, tag="mv")
                nc.vector.bn_aggr(mv[:], st6[:])
                sd = stp.tile([P, 1], FP32, tag="sd")
                nc.scalar.activation(sd[:], mv[:, 1:2], AF.Sqrt, bias=eps_t[:])
                rstd = stp.tile([P, 1], FP32, tag="rstd")
                nc.vector.reciprocal(rstd[:], sd[:])
                if use_g:
                    tmp = wp.tile([P, HID], FP32, tag="lnt")
                    nc.vector.tensor_scalar(tmp[:], src_ap, mv[:, 0:1], rstd[:],
                                            ALU.subtract, ALU.mult)
                    nc.vector.tensor_tensor(out=tmp[:], in0=tmp[:],
                                            in1=gb_t[li][:], op=ALU.mult)
                    nc.vector.tensor_tensor(out=dst_ap, in0=tmp[:],
                                            in1=bb_t[li][:], op=ALU.add)
                else:
                    nc.vector.tensor_scalar(dst_ap, src_ap, mv[:, 0:1], rstd[:],
                                            ALU.subtract, ALU.mult)

            # ================= Layer 0 =================
            # own shard only: h0/hn0 resident; hn0 table built by a split
            # AllGather (A mid-pass, B at end) exactly like the other layers.
            agB_i = {}
            with tc.tile_pool(name="l0pool", bufs=1) as l0p:
                xo = l0p.tile([K0, SH], FP32, tag="xo")
                nc.sync.dma_start(out=xo[:], in_=xTo[:, :])
                for b in range(NB):
                    ps = pmisc.tile([P, HID], FP32, tag="pm")
                    nc.tensor.matmul(out=ps[:], lhsT=xo[:, b * P:(b + 1) * P],
                                     rhs=Wa_t[:], start=True, stop=True)
                    nc.scalar.copy(out=h_blk[b][:], in_=ps[:])
                    layer_norm_tile(ps[:], hn_blk[b][:], 0)
                    if b < 25:
                        nc.sync.dma_start(out=ag_inA[b * P:(b + 1) * P, :],
                                          in_=hn_blk[b][:])
                    else:
                        nc.sync.dma_start(out=ag_inB[(b - 25) * P:(b - 24) * P, :],
                                          in_=hn_blk[b][:])
                nc.gpsimd.collective_compute(
                    "AllGather", mybir.AluOpType.bypass,
                    replica_groups=[list(range(NCORES))],
                    ins=[ag_inA[:, :]], outs=[hn0_d[24576:, :]])
                agB_i[0] = nc.gpsimd.collective_compute(
                    "AllGather", mybir.AluOpType.bypass,
                    replica_groups=[list(range(NCORES))],
                    ins=[ag_inB[:, :]], outs=[hn0_d[:24576, :]])

            # ================= Layers 1..3 =================
            nwin = (TT + WIN - 1) // WIN
            lyr_stack = tc.tile_pool(name="gpool", bufs=8)
            gp = lyr_stack.__enter__()
            sp_cm = tc.tile_pool(name="spool", bufs=4); sp = sp_cm.__enter__()
            fp_cm = tc.tile_pool(name="flush", bufs=4); fp = fp_cm.__enter__()
            for li in range(N_LAYERS):
                table = (hn0_d, ag1_d, ag2_d)[li]
                tab_hi = table[32768:, :]  # signed-idx base
                aggT = {}  # block -> sbuf tile
                for w in range(nwin):
                    t0, t1 = w * WIN, min((w + 1) * WIN, TT)
                    wt = t1 - t0
                    g = gp.tile([P, WIN, HID], FP32, tag="g")
                    g_i = nc.gpsimd.dma_gather(
                        g[:, :wt, :], tab_hi,
                        idx_t[:, t0 * 8:t1 * 8], wt * P, wt * P, HID)
                    if win_b[w]:
                        add_dep_helper(g_i.ins, agB_i[li].ins, sync=True,
                                       reason="window reads B rows: wait for AG-B")
                    for t in range(t0, t1):
                        b = int(tile_block[t])
                        first = (t == 0) or (tile_block[t - 1] != b)
                        last = (t == TT - 1) or (tile_block[t + 1] != b)
                        S = sp.tile([P, P], FP32, tag="S")
                        nc.vector.tensor_scalar(
                            S[:], iota_t[:], dcol_t[:, t:t + 1], vcol_t[:, t:t + 1],
                            ALU.is_equal, ALU.mult)
                        if first:
                            aggT[b] = pagg.tile([P, P], FP32, tag="paggT", name=f"paggT{b}")
                        nc.tensor.matmul(
                            out=aggT[b][:],
                            lhsT=g[:, t - t0, :],
                            rhs=S[:], start=first, stop=last)
                        if last:
                            # flush block b: dense + residual + relu (+ LN)
                            aggT_s = fp.tile([P, P], FP32, tag="aggTs")
                            nc.scalar.copy(out=aggT_s[:], in_=aggT[b][:])
                            ph = pmisc.tile([P, P], FP32, tag="pm")
                            nc.tensor.transpose(out=ph[:], in_=hn_blk[b][:],
                                                identity=ident_t[:])
                            hnT_s = fp.tile([P, P], FP32, tag="hnTs")
                            nc.scalar.copy(out=hnT_s[:], in_=ph[:])
                            pc = pmisc.tile([P, HID], FP32, tag="pm")
                            nc.tensor.matmul(out=pc[:], lhsT=aggT_s[:],
                                             rhs=Wl_t[li][:], start=True, stop=False)
                            nc.tensor.matmul(out=pc[:], lhsT=hnT_s[:],
                                             rhs=Wr_t[li][:], start=False, stop=True)
                            hin = wp.tile([P, HID], FP32, tag="hin")
                            nc.vector.tensor_tensor(out=hin[:], in0=h_blk[b][:],
                                                    in1=pc[:], op=ALU.add)
                            if use_bl:
                                nc.vector.tensor_tensor(out=hin[:], in0=hin[:],
                                                        in1=blb_t[li][:], op=ALU.add)
                            nc.vector.tensor_relu(out=h_blk[b][:], in_=hin[:])
                            if li < N_LAYERS - 1:
                                layer_norm_tile(h_blk[b][:], hn_blk[b][:], li + 1)
                                if b < 25:
                                    nc.sync.dma_start(
                                        out=ag_inA[b * P:(b + 1) * P, :],
                                        in_=hn_blk[b][:])
                                else:
                                    nc.sync.dma_start(
                                        out=ag_inB[(b - 25) * P:(b - 24) * P, :],
                                        in_=hn_blk[b][:])
                            else:
                                ph3 = pmisc.tile([P, P], FP32, tag="pm")
                                nc.tensor.transpose(out=ph3[:], in_=h_blk[b][:],
                                                    identity=ident_t[:])
                                h3T_s = fp.tile([P, P], FP32, tag="h3Ts")
                                nc.scalar.copy(out=h3T_s[:], in_=ph3[:])
                                po = pmisc.tile([P, D_OUT], FP32, tag="pm")
                                nc.tensor.matmul(out=po[:], lhsT=h3T_s[:],
                                                 rhs=Wout_t[:], start=True, stop=True)
                                o_s = wp.tile([P, D_OUT], FP32, tag="outs")
                                if use_bout:
                                    nc.vector.tensor_tensor(out=o_s[:], in0=po[:],
                                                            in1=bob_t[:], op=ALU.add)
                                else:
                                    nc.scalar.copy(out=o_s[:], in_=po[:])
                                nc.sync.dma_start(out=out_d[b * P:(b + 1) * P, :],
                                                  in_=o_s[:])
                if li < N_LAYERS - 1:
                    ag_out = (ag1_d, ag2_d)[li]
                    nc.gpsimd.collective_compute(
                        "AllGather", mybir.AluOpType.bypass,
                        replica_groups=[list(range(NCORES))],
                        ins=[ag_inA[:, :]], outs=[ag_out[24576:, :]])
                    agB_i[li + 1] = nc.gpsimd.collective_compute(
                        "AllGather", mybir.AluOpType.bypass,
                        replica_groups=[list(range(NCORES))],
                        ins=[ag_inB[:, :]], outs=[ag_out[:24576, :]])
            fp_cm.__exit__(None, None, None)
            sp_cm.__exit__(None, None, None)
            lyr_stack.__exit__(None, None, None)

    nc.compile()
    return nc


def _get_runner(inputs):
    key = (hash(np.asarray(inputs["edge_index"]).tobytes()),
           tuple(np.asarray(inputs["x"]).shape))
    if key in _CACHE:
        return _CACHE[key]

    meta = _host_prep(inputs["edge_index"])
    use_bin = bool(np.any(np.asarray(inputs["b_in"]) != 0))
    use_bl = bool(np.any(np.asarray(inputs["bl"]) != 0))
    use_g = bool(np.any(np.asarray(inputs["ln_g"]) != 1.0)
                 or np.any(np.asarray(inputs["ln_b"]) != 0))
    use_bout = bool(np.any(np.asarray(inputs["b_out"]) != 0))
    nc = _build_program(meta, use_bin, use_bl, use_g, use_bout)
    from runner_embedded import SpmdRunner
    runner = SpmdRunner(nc, NCORES)
    _CACHE[key] = (runner, meta, use_bin, use_bl, use_g, use_bout)
    return _CACHE[key]


def kernel(**inputs):
    runner, meta, use_bin, use_bl, use_g, use_bout = _get_runner(inputs)

    x = np.asarray(inputs["x"], np.float32)
    W_in = np.asarray(inputs["W_in"], np.float32)
    b_in = np.asarray(inputs["b_in"], np.float32)
    Wl = np.asarray(inputs["Wl"], np.float32)
    bl = np.asarray(inputs["bl"], np.float32)
    Wr = np.asarray(inputs["Wr"], np.float32)
    ln_g = np.asarray(inputs["ln_g"], np.float32)
    ln_b = np.asarray(inputs["ln_b"], np.float32)
    W_out = np.asarray(inputs["W_out"], np.float32)
    b_out = np.asarray(inputs["b_out"], np.float32)

    x_pad = np.zeros((NP, D_IN), np.float32)
    x_pad[:N_NODES] = x
    xT = x_pad.T  # [16, NP]
    if use_bin:
        xTa = np.concatenate([xT, np.ones((1, NP), np.float32)], axis=0)
        Wa = np.concatenate([W_in, b_in[None, :]], axis=0)
    else:
        xTa, Wa = xT, W_in
    iota = np.tile(np.arange(P, dtype=np.float32)[None, :], (P, 1))
    ident = np.eye(P, dtype=np.float32)

    base = {
        "Wa": np.ascontiguousarray(Wa),
        "Wl": Wl, "Wr": Wr, "Wout": W_out,
        "iota": iota, "ident": ident,
    }
    if use_bl:
        base["blb"] = np.tile(bl[:, None, :], (1, P, 1))
    if use_g:
        base["gb"] = np.tile(ln_g[:, None, :], (1, P, 1))
        base["bb"] = np.tile(ln_b[:, None, :], (1, P, 1))
    if use_bout:
        base["bob"] = np.tile(b_out[None, :], (P, 1))

    in_maps = []
    for c in range(NCORES):
        m = dict(base)
        m["xTo"] = np.ascontiguousarray(xTa[:, c * SH:(c + 1) * SH])
        m["idx16"] = meta["idx16"][c]
        m["dcol"] = meta["dcol"][c]
        m["vcol"] = meta["vcol"][c]
        in_maps.append(m)

    runner.stage(in_maps)
    res = runner.results()
    out = np.concatenate([res[c]["out"] for c in range(NCORES)], axis=0)
    return out[:N_NODES].astype(np.float32)


# ---------------------------------------------------------------------------
# embedded PJRT runner (self-contained; mirrors bass2jax.run_bass_via_pjrt)
import types as _types

_runner_mod = _types.ModuleType("runner_embedded")
_runner_src = '''
import sys
sys.path.insert(0, "/opt/trn_rl_repo")
import numpy as np
import jax
from jax.sharding import Mesh, PartitionSpec, NamedSharding
from jax.experimental.shard_map import shard_map
import concourse.mybir as mybir
from concourse.bass2jax import _bass_exec_p, install_neuronx_cc_hook, partition_id_tensor


class SpmdRunner:
    def __init__(self, nc, n_cores=8):
        install_neuronx_cc_hook()
        self.nc = nc
        self.n_cores = n_cores
        partition_name = nc.partition_id_tensor.name if nc.partition_id_tensor else None
        in_names, out_names, out_avals, zero_outs = [], [], [], []
        for alloc in nc.m.functions[0].allocations:
            if not isinstance(alloc, mybir.MemoryLocationSet):
                continue
            name = alloc.memorylocations[0].name
            if alloc.kind == "ExternalInput":
                if name != partition_name and name != (nc.dbg_addr.name if nc.dbg_addr else None):
                    in_names.append(name)
            elif alloc.kind == "ExternalOutput":
                shape = tuple(alloc.tensor_shape)
                dtype = mybir.dt.np(alloc.dtype)
                out_names.append(name)
                out_avals.append(jax.core.ShapedArray(shape, dtype))
                zero_outs.append(np.zeros(shape, dtype))
        self.in_names, self.out_names = in_names, out_names
        self.out_avals, self.zero_outs = out_avals, zero_outs
        n_params, n_outs = len(in_names), len(out_names)
        self.n_params = n_params
        all_names = list(in_names) + list(out_names)
        if nc.dbg_addr is not None:
            all_names.append(nc.dbg_addr.name)
        if partition_name is not None:
            all_names.append(partition_name)
        has_dbg = nc.dbg_addr is not None

        def _body(*args):
            operands = list(args)
            if has_dbg:
                operands.append(np.zeros((1, 2), np.uint32))
            if partition_name is not None:
                operands.append(partition_id_tensor())
            outs = _bass_exec_p.bind(
                *operands,
                out_avals=tuple(out_avals),
                in_names=tuple(all_names),
                out_names=tuple(out_names),
                lowering_input_output_aliases=(),
                sim_require_finite=True,
                sim_require_nnan=True,
                nc=nc,
            )
            return tuple(outs)

        devices = jax.devices()[:n_cores]
        self.mesh = Mesh(np.asarray(devices), ("core",))
        self.sharding = NamedSharding(self.mesh, PartitionSpec("core"))
        in_specs = (PartitionSpec("core"),) * (n_params + n_outs)
        out_specs = (PartitionSpec("core"),) * n_outs
        self.fn = jax.jit(
            shard_map(_body, mesh=self.mesh, in_specs=in_specs,
                      out_specs=out_specs, check_rep=False),
            keep_unused=True,
        )
        self.dev_in = None

    def stage(self, in_maps):
        per_core = [[np.asarray(m[n]) for n in self.in_names] for m in in_maps]
        concat_in = [
            np.concatenate([per_core[c][i] for c in range(self.n_cores)], axis=0)
            for i in range(self.n_params)
        ]
        concat_zero = [
            np.zeros((self.n_cores * z.shape[0], *z.shape[1:]), z.dtype)
            for z in self.zero_outs
        ]
        self.dev_in = [jax.device_put(a, self.sharding) for a in concat_in + concat_zero]
        return self

    def run(self):
        outs = self.fn(*self.dev_in)
        jax.block_until_ready(outs)
        return outs

    def results(self):
        outs = self.run()
        return [
            {name: np.asarray(outs[i]).reshape(self.n_cores, *self.out_avals[i].shape)[c]
             for i, name in enumerate(self.out_names)}
            for c in range(self.n_cores)
        ]
'''
exec(compile(_runner_src, "runner_embedded", "exec"), _runner_mod.__dict__)
sys.modules["runner_embedded"] = _runner_mod


# revision 18
# speedup vs baseline: 1.1538x; 1.1538x over previous
"""DropEdge GraphSAGE (eval mode) on 8 Trainium2 NeuronCores.

Strategy (graph/data parallel, per sharding hint):
- Nodes padded 50000->50176 = 8 cores x 6272 (49 blocks of 128).
- Edges sharded by destination core; per core grouped by (dst block, src
  parity), sorted by src, padded to 128-edge tiles (uniform tile counts
  across cores so the SPMD program is identical).
- Layer 0 (x @ W_in + LN): computed fully on every core (replicated) to
  avoid an AllGather; own-shard h/hn also kept in SBUF.
- Aggregation per layer: indirect DMA gather of *row pairs* (1024B
  descriptors, idx = src>>1 fits int16) from the replicated hn table in
  DRAM; one-hot selection matrices S (built on DVE from iota vs dst ids,
  with 1/deg folded in) turn segment-sum into PE matmuls accumulating in
  PSUM per dst block, feature-major: aggT = G^T S.
- Dense part: conv = aggT^T Wl + hnT^T Wr in PSUM; residual+relu; LN via
  bn_stats/bn_aggr; AllGather of the new hn shard between layers.
- Final layer: h3^T W_out -> per-core [6272,4]; host concatenates + trims.
"""
import sys
sys.path.insert(0, "/opt/trn_rl_repo")
import numpy as np

N_NODES = 50000
N_EDGES = 800000
D_IN = 16
HID = 128
D_OUT = 4
N_LAYERS = 3
LN_EPS = 1e-5

P = 128
NCORES = 8
NP = 50176            # padded nodes
SH = NP // NCORES     # 6272 per core
NB = SH // P          # 49 blocks per core
NT_FULL = NP // P     # 392 node tiles
WIN = 8               # tiles per gather window (<=1024 idxs)

_CACHE = {}


def _host_prep(edge_index):
    src = np.asarray(edge_index[0], dtype=np.int64)
    dst = np.asarray(edge_index[1], dtype=np.int64)
    deg = np.bincount(dst, minlength=NP).astype(np.float32)
    inv_deg = 1.0 / np.maximum(deg, 1.0)

    # Per-core balanced repacking: each core assigns its own 6272 nodes to
    # its 49 blocks so block in-degree sums are flat (~2041 < 2048 = 16
    # tiles); blocks then sorted by sum desc so the shared per-local-index
    # tile counts are minimal. newpos[n] = new global slot of node n.
    import heapq
    newpos = np.empty(NP, np.int64)
    for c in range(NCORES):
        nodes_c = np.arange(c * SH, min((c + 1) * SH, NP))
        degs_c = deg[nodes_c]
        o_ = np.argsort(-degs_c, kind="stable")
        heap = [(0.0, 0, b_) for b_ in range(NB)]
        heapq.heapify(heap)
        blocks = [[] for _ in range(NB)]
        sums = np.zeros(NB)
        for i_ in o_:
            while True:
                s_, _, b_ = heapq.heappop(heap)
                if len(blocks[b_]) < P:
                    break
            blocks[b_].append(nodes_c[i_])
            sums[b_] += degs_c[i_]
            if len(blocks[b_]) < P:
                heapq.heappush(heap, (sums[b_], len(blocks[b_]), b_))
        border = np.argsort(-sums, kind="stable")  # largest block -> local 0
        for bl, g in enumerate(border):
            for sl, n_ in enumerate(blocks[g]):
                newpos[n_] = c * SH + bl * P + sl

    src_n = newpos[src]
    dst_n = newpos[dst]
    core = dst_n // SH
    block = (dst_n % SH) // P
    order = np.lexsort((src_n, block, core))
    s_src, s_dst, s_core, s_blk = src_n[order], dst_n[order], core[order], block[order]
    s_dst_orig = dst[order]  # for inv_deg lookup (deg is per original node)
    key = s_core * NB + s_blk
    cnt = np.bincount(key, minlength=NCORES * NB).reshape(NCORES, NB)
    # uniform tiles per block = max over cores (>=1 so PSUM is initialized)
    tiles_b = np.maximum(np.ceil(cnt.max(axis=0) / P).astype(np.int64), 1)  # [NB]
    TT = int(tiles_b.sum())

    tile_block = np.zeros(TT, np.int64)
    seg_off = np.zeros(NB, np.int64)
    t = 0
    for b in range(NB):
        seg_off[b] = t
        n = int(tiles_b[b])
        tile_block[t:t + n] = b
        t += n
    assert t == TT

    # Table rows are PERMUTED: each core's blocks 0..24 (the "A" half,
    # all-gathered mid-layer) land in high rows [24576, 50176); blocks
    # 25..48 ("B", all-gathered at layer end) land in [0, 24576). The
    # signed-idx gather AP starts at 32768 (inside A), so Tile auto-deps
    # gathers on the A collective; B gets explicit deps per window.
    nc_ = np.arange(NP) // SH
    nr_ = np.arange(NP) % SH
    nblk = nr_ // P
    rowmap = np.where(
        nblk < 25,
        24576 + nc_ * 3200 + nr_,
        nc_ * 3072 + (nr_ - 25 * P))
    PAD_IDX = 50175 - 32768  # an A-region row; harmless (S column zero)
    pidx = np.full((NCORES, TT * P), PAD_IDX, np.int64)
    dcol = np.full((NCORES, TT * P), -1.0, np.float32)
    vcol = np.zeros((NCORES, TT * P), np.float32)
    starts = np.concatenate([[0], np.cumsum(cnt.reshape(-1))])
    for c in range(NCORES):
        for b in range(NB):
            k = c * NB + b
            lo, hi = starts[k], starts[k + 1]
            n = hi - lo
            if n == 0:
                continue
            off = seg_off[b] * P
            rows = rowmap[s_src[lo:hi]]
            o2 = np.argsort(-rows, kind="stable")  # A-rows (high) first
            pidx[c, off:off + n] = rows[o2] - 32768
            dcol[c, off:off + n] = ((s_dst[lo:hi] % SH) % P)[o2]
            vcol[c, off:off + n] = inv_deg[s_dst_orig[lo:hi]][o2]

    # Q7 drops trailing-negative idxs per call: the LAST idx of every
    # 1024-idx gather window must be >= 0. Swap a non-negative idx (high
    # src or pad) from the same segment (same dst block -> semantics
    # unchanged) into each bad window's last slot.
    nwin = (TT + WIN - 1) // WIN
    win_last = set((min((w + 1) * WIN, TT) * P) - 1 for w in range(nwin))
    seg_lo = seg_off * P                      # edge offset of each segment
    seg_hi = (seg_off + tiles_b) * P
    for c in range(NCORES):
        for w in range(nwin):
            e1 = min((w + 1) * WIN, TT) * P
            j = e1 - 1
            if pidx[c, j] >= 0:
                continue
            b = int(tile_block[(e1 - 1) // P])
            cand = seg_lo[b] + np.nonzero(pidx[c, seg_lo[b]:seg_hi[b]] >= 0)[0]
            cand = [int(q) for q in cand if int(q) not in win_last]
            if not cand:
                raise ValueError("segment with no high-src edge or pad; "
                                 "unsupported input distribution")
            q = cand[-1]
            for arr in (pidx, dcol, vcol):
                arr[c, q], arr[c, j] = arr[c, j], arr[c, q]

    # classify windows AFTER swaps: window needs the B collective iff any
    # core's window touches a row < 24576 (same flag on all cores - SPMD)
    win_b = np.zeros(nwin, bool)
    for w in range(nwin):
        e0, e1 = w * WIN * P, min((w + 1) * WIN, TT) * P
        win_b[w] = bool((pidx[:, e0:e1] + 32768 < 24576).any())

    idx16 = np.tile(
        pidx.astype(np.int16).reshape(NCORES, TT * P // 16, 16).transpose(0, 2, 1),
        (1, 8, 1))  # [NCORES, 128, TT*8]
    dcol = dcol.reshape(NCORES, TT, P).transpose(0, 2, 1)
    vcol = vcol.reshape(NCORES, TT, P).transpose(0, 2, 1)

    return dict(idx16=idx16, dcol=np.ascontiguousarray(dcol),
                vcol=np.ascontiguousarray(vcol),
                tiles_b=tiles_b, tile_block=tile_block, TT=TT, win_b=win_b,
                newpos=newpos)


def _build_program(meta, use_bin, use_bl, use_g, use_bout):
    import concourse.bacc as bacc
    import concourse.mybir as mybir
    import concourse.tile as tile
    from concourse.tile_rust import add_dep_helper
    from concourse.alu_op_type import AluOpType as ALU

    FP32 = mybir.dt.float32
    I16 = mybir.dt.int16
    AF = mybir.ActivationFunctionType

    TT = meta["TT"]
    tile_block = meta["tile_block"]
    win_b = meta["win_b"]
    K0 = D_IN + (1 if use_bin else 0)

    nc = bacc.Bacc("TRN2", target_bir_lowering=False, debug=False,
                   num_devices=NCORES)

    # ---- I/O ----
    xTo = nc.dram_tensor("xTo", [K0, SH], FP32, kind="ExternalInput")
    Wa = nc.dram_tensor("Wa", [K0, HID], FP32, kind="ExternalInput")
    Wl = nc.dram_tensor("Wl", [N_LAYERS, HID, HID], FP32, kind="ExternalInput")
    Wr = nc.dram_tensor("Wr", [N_LAYERS, HID, HID], FP32, kind="ExternalInput")
    Wout = nc.dram_tensor("Wout", [HID, D_OUT], FP32, kind="ExternalInput")
    iota_d = nc.dram_tensor("iota", [P, P], FP32, kind="ExternalInput")
    ident_d = nc.dram_tensor("ident", [P, P], FP32, kind="ExternalInput")
    idx_d = nc.dram_tensor("idx16", [P, TT * 8], I16, kind="ExternalInput")
    dcol_d = nc.dram_tensor("dcol", [P, TT], FP32, kind="ExternalInput")
    vcol_d = nc.dram_tensor("vcol", [P, TT], FP32, kind="ExternalInput")
    if use_bl:
        blb_d = nc.dram_tensor("blb", [N_LAYERS, P, HID], FP32, kind="ExternalInput")
    if use_g:
        gb_d = nc.dram_tensor("gb", [N_LAYERS, P, HID], FP32, kind="ExternalInput")
        bb_d = nc.dram_tensor("bb", [N_LAYERS, P, HID], FP32, kind="ExternalInput")
    if use_bout:
        bob_d = nc.dram_tensor("bob", [P, D_OUT], FP32, kind="ExternalInput")
    out_d = nc.dram_tensor("out", [SH, D_OUT], FP32, kind="ExternalOutput")

    # ---- internal DRAM ----
    hn0_d = nc.dram_tensor("hn0", [NP, HID], FP32, addr_space="Shared")
    ag_inA = nc.dram_tensor("ag_inA", [25 * P, HID], FP32)
    ag_inB = nc.dram_tensor("ag_inB", [24 * P, HID], FP32)
    ag1_d = nc.dram_tensor("ag1", [NP, HID], FP32, addr_space="Shared")
    ag2_d = nc.dram_tensor("ag2", [NP, HID], FP32, addr_space="Shared")

    with tile.TileContext(nc) as tc:
        with (
            tc.tile_pool(name="const", bufs=1) as cp,
            tc.tile_pool(name="resid", bufs=1) as rp,
            tc.tile_pool(name="work", bufs=4) as wp,
            tc.tile_pool(name="stat", bufs=4) as stp,
            tc.tile_pool(name="pagg", bufs=3, space="PSUM") as pagg,
            tc.tile_pool(name="pmisc", bufs=4, space="PSUM") as pmisc,
        ):
            # ---- constants into SBUF ----
            iota_t = cp.tile([P, P], FP32)
            nc.sync.dma_start(out=iota_t[:], in_=iota_d[:, :])
            ident_t = cp.tile([P, P], FP32)
            nc.sync.dma_start(out=ident_t[:], in_=ident_d[:, :])
            Wa_t = cp.tile([K0, HID], FP32)
            nc.sync.dma_start(out=Wa_t[:], in_=Wa[:, :])
            Wl_t = [cp.tile([HID, HID], FP32, tag=f"wl{i}", name=f"wl{i}") for i in range(3)]
            Wr_t = [cp.tile([HID, HID], FP32, tag=f"wr{i}", name=f"wr{i}") for i in range(3)]
            for i in range(3):
                nc.sync.dma_start(out=Wl_t[i][:], in_=Wl[i, :, :])
                nc.sync.dma_start(out=Wr_t[i][:], in_=Wr[i, :, :])
            Wout_t = cp.tile([HID, D_OUT], FP32)
            nc.sync.dma_start(out=Wout_t[:], in_=Wout[:, :])
            idx_t = cp.tile([P, TT * 8], I16)
            nc.sync.dma_start(out=idx_t[:], in_=idx_d[:, :])
            dcol_t = cp.tile([P, TT], FP32)
            nc.sync.dma_start(out=dcol_t[:], in_=dcol_d[:, :])
            vcol_t = cp.tile([P, TT], FP32)
            nc.sync.dma_start(out=vcol_t[:], in_=vcol_d[:, :])
            if use_bl:
                blb_t = [cp.tile([P, HID], FP32, tag=f"blb{i}", name=f"blb{i}") for i in range(3)]
                for i in range(3):
                    nc.sync.dma_start(out=blb_t[i][:], in_=blb_d[i, :, :])
            if use_g:
                gb_t = [cp.tile([P, HID], FP32, tag=f"gb{i}", name=f"gb{i}") for i in range(3)]
                bb_t = [cp.tile([P, HID], FP32, tag=f"bb{i}", name=f"bb{i}") for i in range(3)]
                for i in range(3):
                    nc.sync.dma_start(out=gb_t[i][:], in_=gb_d[i, :, :])
                    nc.sync.dma_start(out=bb_t[i][:], in_=bb_d[i, :, :])
            if use_bout:
                bob_t = cp.tile([P, D_OUT], FP32)
                nc.sync.dma_start(out=bob_t[:], in_=bob_d[:, :])

            eps_t = cp.tile([P, 1], FP32)
            nc.vector.memset(eps_t[:], LN_EPS)
            h_blk = [rp.tile([P, HID], FP32, tag=f"h{b}", name=f"h{b}") for b in range(NB)]
            hn_blk = [rp.tile([P, HID], FP32, tag=f"hn{b}", name=f"hn{b}") for b in range(NB)]

            def layer_norm_tile(src_ap, dst_ap, li):
                """dst = LN(src) (optionally *g+b). src may be PSUM."""
                st6 = stp.tile([P, 6], FP32, tag="st6")
                nc.vector.bn_stats(st6[:], src_ap)
                mv = stp.tile([P, 2], FP32, tag="mv")
                nc.vector.bn_aggr(mv[:], st6[:])
                sd = stp.tile([P, 1], FP32, tag="sd")
                nc.scalar.activation(sd[:], mv[:, 1:2], AF.Sqrt, bias=eps_t[:])
                rstd = stp.tile([P, 1], FP32, tag="rstd")
                nc.vector.reciprocal(rstd[:], sd[:])
                if use_g:
                    tmp = wp.tile([P, HID], FP32, tag="lnt")
                    nc.vector.tensor_scalar(tmp[:], src_ap, mv[:, 0:1], rstd[:],
                                            ALU.subtract, ALU.mult)
                    nc.vector.tensor_tensor(out=tmp[:], in0=tmp[:],
                                            in1=gb_t[li][:], op=ALU.mult)
                    nc.vector.tensor_tensor(out=dst_ap, in0=tmp[:],
                                            in1=bb_t[li][:], op=ALU.add)
                else:
                    nc.vector.tensor_scalar(dst_ap, src_ap, mv[:, 0:1], rstd[:],
                                            ALU.subtract, ALU.mult)

            # ================= Layer 0 =================
            # own shard only: h0/hn0 resident; hn0 table built by a split
            # AllGather (A mid-pass, B at end) exactly like the other layers.
            agB_i = {}
            with tc.tile_pool(name="l0pool", bufs=1) as l0p:
                xo = l0p.tile([K0, SH], FP32, tag="xo")
                nc.sync.dma_start(out=xo[:], in_=xTo[:, :])
                for b in range(NB):
                    ps = pmisc.tile([P, HID], FP32, tag="pm")
                    nc.tensor.matmul(out=ps[:], lhsT=xo[:, b * P:(b + 1) * P],
                                     rhs=Wa_t[:], start=True, stop=True)
                    nc.scalar.copy(out=h_blk[b][:], in_=ps[:])
                    layer_norm_tile(ps[:], hn_blk[b][:], 0)
                    if b < 25:
                        nc.sync.dma_start(out=ag_inA[b * P:(b + 1) * P, :],
                                          in_=hn_blk[b][:])
                    else:
                        nc.sync.dma_start(out=ag_inB[(b - 25) * P:(b - 24) * P, :],
                                          in_=hn_blk[b][:])
                nc.gpsimd.collective_compute(
                    "AllGather", mybir.AluOpType.bypass,
                    replica_groups=[list(range(NCORES))],
                    ins=[ag_inA[:, :]], outs=[hn0_d[24576:, :]])
                agB_i[0] = nc.gpsimd.collective_compute(
                    "AllGather", mybir.AluOpType.bypass,
                    replica_groups=[list(range(NCORES))],
                    ins=[ag_inB[:, :]], outs=[hn0_d[:24576, :]])

            # ================= Layers 1..3 =================
            nwin = (TT + WIN - 1) // WIN
            lyr_stack = tc.tile_pool(name="gpool", bufs=8)
            gp = lyr_stack.__enter__()
            sp_cm = tc.tile_pool(name="spool", bufs=4); sp = sp_cm.__enter__()
            fp_cm = tc.tile_pool(name="flush", bufs=4); fp = fp_cm.__enter__()
            for li in range(N_LAYERS):
                table = (hn0_d, ag1_d, ag2_d)[li]
                tab_hi = table[32768:, :]  # signed-idx base
                aggT = {}  # block -> sbuf tile
                for w in range(nwin):
                    t0, t1 = w * WIN, min((w + 1) * WIN, TT)
                    wt = t1 - t0
                    g = gp.tile([P, WIN, HID], FP32, tag="g")
                    g_i = nc.gpsimd.dma_gather(
                        g[:, :wt, :], tab_hi,
                        idx_t[:, t0 * 8:t1 * 8], wt * P, wt * P, HID)
                    if win_b[w]:
                        add_dep_helper(g_i.ins, agB_i[li].ins, sync=True,
                                       reason="window reads B rows: wait for AG-B")
                    for t in range(t0, t1):
                        b = int(tile_block[t])
                        first = (t == 0) or (tile_block[t - 1] != b)
                        last = (t == TT - 1) or (tile_block[t + 1] != b)
                        S = sp.tile([P, P], FP32, tag="S")
                        nc.vector.tensor_scalar(
                            S[:], iota_t[:], dcol_t[:, t:t + 1], vcol_t[:, t:t + 1],
                            ALU.is_equal, ALU.mult)
                        if first:
                            aggT[b] = pagg.tile([P, P], FP32, tag="paggT", name=f"paggT{b}")
                        nc.tensor.matmul(
                            out=aggT[b][:],
                            lhsT=g[:, t - t0, :],
                            rhs=S[:], start=first, stop=last)
                        if last:
                            # flush block b: dense + residual + relu (+ LN)
                            aggT_s = fp.tile([P, P], FP32, tag="aggTs")
                            nc.scalar.copy(out=aggT_s[:], in_=aggT[b][:])
                            ph = pmisc.tile([P, P], FP32, tag="pm")
                            nc.tensor.transpose(out=ph[:], in_=hn_blk[b][:],
                                                identity=ident_t[:])
                            hnT_s = fp.tile([P, P], FP32, tag="hnTs")
                            nc.scalar.copy(out=hnT_s[:], in_=ph[:])
                            pc = pmisc.tile([P, HID], FP32, tag="pm")
                            nc.tensor.matmul(out=pc[:], lhsT=aggT_s[:],
                                             rhs=Wl_t[li][:], start=True, stop=False)
                            nc.tensor.matmul(out=pc[:], lhsT=hnT_s[:],
                                             rhs=Wr_t[li][:], start=False, stop=True)
                            hin = wp.tile([P, HID], FP32, tag="hin")
                            nc.vector.tensor_tensor(out=hin[:], in0=h_blk[b][:],
                                                    in1=pc[:], op=ALU.add)
                            if use_bl:
                                nc.vector.tensor_tensor(out=hin[:], in0=hin[:],
                                                        in1=blb_t[li][:], op=ALU.add)
                            nc.vector.tensor_relu(out=h_blk[b][:], in_=hin[:])
                            if li < N_LAYERS - 1:
                                layer_norm_tile(h_blk[b][:], hn_blk[b][:], li + 1)
                                if b < 25:
                                    nc.sync.dma_start(
                                        out=ag_inA[b * P:(b + 1) * P, :],
                                        in_=hn_blk[b][:])
                                else:
                                    nc.sync.dma_start(
                                        out=ag_inB[(b - 25) * P:(b - 24) * P, :],
                                        in_=hn_blk[b][:])
                            else:
                                ph3 = pmisc.tile([P, P], FP32, tag="pm")
                                nc.tensor.transpose(out=ph3[:], in_=h_blk[b][:],
                                                    identity=ident_t[:])
                                h3T_s = fp.tile([P, P], FP32, tag="h3Ts")
                                nc.scalar.copy(out=h3T_s[:], in_=ph3[:])
                                po = pmisc.tile([P, D_OUT], FP32, tag="pm")
                                nc.tensor.matmul(out=po[:], lhsT=h3T_s[:],
                                                 rhs=Wout_t[:], start=True, stop=True)
                                o_s = wp.tile([P, D_OUT], FP32, tag="outs")
                                if use_bout:
                                    nc.vector.tensor_tensor(out=o_s[:], in0=po[:],
                                                            in1=bob_t[:], op=ALU.add)
                                else:
                                    nc.scalar.copy(out=o_s[:], in_=po[:])
                                nc.sync.dma_start(out=out_d[b * P:(b + 1) * P, :],
                                                  in_=o_s[:])
                if li < N_LAYERS - 1:
                    ag_out = (ag1_d, ag2_d)[li]
                    nc.gpsimd.collective_compute(
                        "AllGather", mybir.AluOpType.bypass,
                        replica_groups=[list(range(NCORES))],
                        ins=[ag_inA[:, :]], outs=[ag_out[24576:, :]])
                    agB_i[li + 1] = nc.gpsimd.collective_compute(
                        "AllGather", mybir.AluOpType.bypass,
                        replica_groups=[list(range(NCORES))],
                        ins=[ag_inB[:, :]], outs=[ag_out[:24576, :]])
            fp_cm.__exit__(None, None, None)
            sp_cm.__exit__(None, None, None)
            lyr_stack.__exit__(None, None, None)

    nc.compile()
    return nc


def _get_runner(inputs):
    key = (hash(np.asarray(inputs["edge_index"]).tobytes()),
           tuple(np.asarray(inputs["x"]).shape))
    if key in _CACHE:
        return _CACHE[key]

    meta = _host_prep(inputs["edge_index"])
    use_bin = bool(np.any(np.asarray(inputs["b_in"]) != 0))
    use_bl = bool(np.any(np.asarray(inputs["bl"]) != 0))
    use_g = bool(np.any(np.asarray(inputs["ln_g"]) != 1.0)
                 or np.any(np.asarray(inputs["ln_b"]) != 0))
    use_bout = bool(np.any(np.asarray(inputs["b_out"]) != 0))
    nc = _build_program(meta, use_bin, use_bl, use_g, use_bout)
    from runner_embedded import SpmdRunner
    runner = SpmdRunner(nc, NCORES)
    _CACHE[key] = (runner, meta, use_bin, use_bl, use_g, use_bout)
    return _CACHE[key]


def kernel(**inputs):
    runner, meta, use_bin, use_bl, use_g, use_bout = _get_runner(inputs)

    x = np.asarray(inputs["x"], np.float32)
    W_in = np.asarray(inputs["W_in"], np.float32)
    b_in = np.asarray(inputs["b_in"], np.float32)
    Wl = np.asarray(inputs["Wl"], np.float32)
    bl = np.asarray(inputs["bl"], np.float32)
    Wr = np.asarray(inputs["Wr"], np.float32)
    ln_g = np.asarray(inputs["ln_g"], np.float32)
    ln_b = np.asarray(inputs["ln_b"], np.float32)
    W_out = np.asarray(inputs["W_out"], np.float32)
    b_out = np.asarray(inputs["b_out"], np.float32)

    x_pad = np.zeros((NP, D_IN), np.float32)
    x_pad[meta["newpos"][:N_NODES]] = x[:N_NODES] if len(x) >= N_NODES else x
    x_pad[meta["newpos"][N_NODES:]] = 0.0
    xT = x_pad.T  # [16, NP] in permuted slot order
    if use_bin:
        xTa = np.concatenate([xT, np.ones((1, NP), np.float32)], axis=0)
        Wa = np.concatenate([W_in, b_in[None, :]], axis=0)
    else:
        xTa, Wa = xT, W_in
    iota = np.tile(np.arange(P, dtype=np.float32)[None, :], (P, 1))
    ident = np.eye(P, dtype=np.float32)

    base = {
        "Wa": np.ascontiguousarray(Wa),
        "Wl": Wl, "Wr": Wr, "Wout": W_out,
        "iota": iota, "ident": ident,
    }
    if use_bl:
        base["blb"] = np.tile(bl[:, None, :], (1, P, 1))
    if use_g:
        base["gb"] = np.tile(ln_g[:, None, :], (1, P, 1))
        base["bb"] = np.tile(ln_b[:, None, :], (1, P, 1))
    if use_bout:
        base["bob"] = np.tile(b_out[None, :], (P, 1))

    in_maps = []
    for c in range(NCORES):
        m = dict(base)
        m["xTo"] = np.ascontiguousarray(xTa[:, c * SH:(c + 1) * SH])
        m["idx16"] = meta["idx16"][c]
        m["dcol"] = meta["dcol"][c]
        m["vcol"] = meta["vcol"][c]
        in_maps.append(m)

    runner.stage(in_maps)
    res = runner.results()
    out_new = np.concatenate([res[c]["out"] for c in range(NCORES)], axis=0)
    return out_new[meta["newpos"][:N_NODES]].astype(np.float32)


# ---------------------------------------------------------------------------
# embedded PJRT runner (self-contained; mirrors bass2jax.run_bass_via_pjrt)
import types as _types

_runner_mod = _types.ModuleType("runner_embedded")
_runner_src = '''
import sys
sys.path.insert(0, "/opt/trn_rl_repo")
import numpy as np
import jax
from jax.sharding import Mesh, PartitionSpec, NamedSharding
from jax.experimental.shard_map import shard_map
import concourse.mybir as mybir
from concourse.bass2jax import _bass_exec_p, install_neuronx_cc_hook, partition_id_tensor


class SpmdRunner:
    def __init__(self, nc, n_cores=8):
        install_neuronx_cc_hook()
        self.nc = nc
        self.n_cores = n_cores
        partition_name = nc.partition_id_tensor.name if nc.partition_id_tensor else None
        in_names, out_names, out_avals, zero_outs = [], [], [], []
        for alloc in nc.m.functions[0].allocations:
            if not isinstance(alloc, mybir.MemoryLocationSet):
                continue
            name = alloc.memorylocations[0].name
            if alloc.kind == "ExternalInput":
                if name != partition_name and name != (nc.dbg_addr.name if nc.dbg_addr else None):
                    in_names.append(name)
            elif alloc.kind == "ExternalOutput":
                shape = tuple(alloc.tensor_shape)
                dtype = mybir.dt.np(alloc.dtype)
                out_names.append(name)
                out_avals.append(jax.core.ShapedArray(shape, dtype))
                zero_outs.append(np.zeros(shape, dtype))
        self.in_names, self.out_names = in_names, out_names
        self.out_avals, self.zero_outs = out_avals, zero_outs
        n_params, n_outs = len(in_names), len(out_names)
        self.n_params = n_params
        all_names = list(in_names) + list(out_names)
        if nc.dbg_addr is not None:
            all_names.append(nc.dbg_addr.name)
        if partition_name is not None:
            all_names.append(partition_name)
        has_dbg = nc.dbg_addr is not None

        def _body(*args):
            operands = list(args)
            if has_dbg:
                operands.append(np.zeros((1, 2), np.uint32))
            if partition_name is not None:
                operands.append(partition_id_tensor())
            outs = _bass_exec_p.bind(
                *operands,
                out_avals=tuple(out_avals),
                in_names=tuple(all_names),
                out_names=tuple(out_names),
                lowering_input_output_aliases=(),
                sim_require_finite=True,
                sim_require_nnan=True,
                nc=nc,
            )
            return tuple(outs)

        devices = jax.devices()[:n_cores]
        self.mesh = Mesh(np.asarray(devices), ("core",))
        self.sharding = NamedSharding(self.mesh, PartitionSpec("core"))
        in_specs = (PartitionSpec("core"),) * (n_params + n_outs)
        out_specs = (PartitionSpec("core"),) * n_outs
        self.fn = jax.jit(
            shard_map(_body, mesh=self.mesh, in_specs=in_specs,
                      out_specs=out_specs, check_rep=False),
            keep_unused=True,
        )
        self.dev_in = None

    def stage(self, in_maps):
        per_core = [[np.asarray(m[n]) for n in self.in_names] for m in in_maps]
        concat_in = [
            np.concatenate([per_core[c][i] for c in range(self.n_cores)], axis=0)
            for i in range(self.n_params)
        ]
        concat_zero = [
            np.zeros((self.n_cores * z.shape[0], *z.shape[1:]), z.dtype)
            for z in self.zero_outs
        ]
        self.dev_in = [jax.device_put(a, self.sharding) for a in concat_in + concat_zero]
        return self

    def run(self):
        outs = self.fn(*self.dev_in)
        jax.block_until_ready(outs)
        return outs

    def results(self):
        outs = self.run()
        return [
            {name: np.asarray(outs[i]).reshape(self.n_cores, *self.out_avals[i].shape)[c]
             for i, name in enumerate(self.out_names)}
            for c in range(self.n_cores)
        ]
'''
exec(compile(_runner_src, "runner_embedded", "exec"), _runner_mod.__dict__)
sys.modules["runner_embedded"] = _runner_mod


# revision 19
# speedup vs baseline: 1.5630x; 1.3547x over previous
"""DropEdge GraphSAGE (eval mode) on 8 Trainium2 NeuronCores.

Strategy (graph/data parallel, per sharding hint):
- Nodes padded 50000->50176 = 8 cores x 6272 (49 blocks of 128).
- Edges sharded by destination core; per core grouped by (dst block, src
  parity), sorted by src, padded to 128-edge tiles (uniform tile counts
  across cores so the SPMD program is identical).
- Layer 0 (x @ W_in + LN): computed fully on every core (replicated) to
  avoid an AllGather; own-shard h/hn also kept in SBUF.
- Aggregation per layer: indirect DMA gather of *row pairs* (1024B
  descriptors, idx = src>>1 fits int16) from the replicated hn table in
  DRAM; one-hot selection matrices S (built on DVE from iota vs dst ids,
  with 1/deg folded in) turn segment-sum into PE matmuls accumulating in
  PSUM per dst block, feature-major: aggT = G^T S.
- Dense part: conv = aggT^T Wl + hnT^T Wr in PSUM; residual+relu; LN via
  bn_stats/bn_aggr; AllGather of the new hn shard between layers.
- Final layer: h3^T W_out -> per-core [6272,4]; host concatenates + trims.
"""
import sys
sys.path.insert(0, "/opt/trn_rl_repo")
import numpy as np

N_NODES = 50000
N_EDGES = 800000
D_IN = 16
HID = 128
D_OUT = 4
N_LAYERS = 3
LN_EPS = 1e-5

P = 128
NCORES = 8
NP = 50176            # padded nodes
SH = NP // NCORES     # 6272 per core
NB = SH // P          # 49 blocks per core
NT_FULL = NP // P     # 392 node tiles
WIN = 8               # tiles per gather window (<=1024 idxs)

_CACHE = {}


def _host_prep(edge_index):
    src = np.asarray(edge_index[0], dtype=np.int64)
    dst = np.asarray(edge_index[1], dtype=np.int64)
    deg = np.bincount(dst, minlength=NP).astype(np.float32)
    inv_deg = 1.0 / np.maximum(deg, 1.0)

    # Per-core balanced repacking: each core assigns its own 6272 nodes to
    # its 49 blocks so block in-degree sums are flat (~2041 < 2048 = 16
    # tiles); blocks then sorted by sum desc so the shared per-local-index
    # tile counts are minimal. newpos[n] = new global slot of node n.
    import heapq
    CAP = 2047          # 16-tile capacity with margin
    REG_TARGET = 2040   # regular-block mean target
    newpos = np.empty(NP, np.int64)
    for c in range(NCORES):
        nodes_c = np.arange(c * SH, min((c + 1) * SH, NP))
        degs_c = deg[nodes_c]
        o_ = list(np.argsort(-degs_c, kind="stable"))
        total_c = float(degs_c.sum())
        # spill block: seed with highest-degree nodes until the rest fits
        # 48 regular blocks at <= REG_TARGET mean; pad with lowest-degree.
        spill_need = max(total_c - 48 * REG_TARGET, 0.0)
        blk0, s0 = [], 0.0
        while s0 < spill_need and len(blk0) < P and o_:
            i_ = o_.pop(0)
            blk0.append(nodes_c[i_]); s0 += degs_c[i_]
        while len(blk0) < P and o_:
            i_ = o_.pop()            # lowest degree from the tail
            blk0.append(nodes_c[i_]); s0 += degs_c[i_]
        # balance the rest over 48 blocks, capped at CAP
        heap = [(0.0, 0, b_) for b_ in range(48)]
        heapq.heapify(heap)
        blocks = [[] for _ in range(48)]
        sums = np.zeros(48)
        for i_ in o_:
            popped = []
            placed = False
            while heap:
                s_, _, b_ = heapq.heappop(heap)
                if len(blocks[b_]) >= P:
                    continue
                if s_ + degs_c[i_] <= CAP or not placed:
                    if s_ + degs_c[i_] <= CAP:
                        blocks[b_].append(nodes_c[i_])
                        sums[b_] += degs_c[i_]
                        heapq.heappush(heap, (sums[b_], len(blocks[b_]), b_))
                        placed = True
                        break
                popped.append((s_, _, b_))
            for e_ in popped:
                heapq.heappush(heap, e_)
            if not placed:
                # forced: lowest-sum block with space
                b_ = min((b for b in range(48) if len(blocks[b]) < P),
                         key=lambda b: sums[b])
                blocks[b_].append(nodes_c[i_]); sums[b_] += degs_c[i_]
                heapq.heappush(heap, (sums[b_], len(blocks[b_]), b_))
        all_blocks = [blk0] + blocks
        all_sums = np.concatenate([[s0], sums])
        border = np.argsort(-all_sums, kind="stable")
        for bl, g in enumerate(border):
            for sl, n_ in enumerate(all_blocks[g]):
                newpos[n_] = c * SH + bl * P + sl

    src_n = newpos[src]
    dst_n = newpos[dst]
    core = dst_n // SH
    block = (dst_n % SH) // P
    order = np.lexsort((src_n, block, core))
    s_src, s_dst, s_core, s_blk = src_n[order], dst_n[order], core[order], block[order]
    s_dst_orig = dst[order]  # for inv_deg lookup (deg is per original node)
    key = s_core * NB + s_blk
    cnt = np.bincount(key, minlength=NCORES * NB).reshape(NCORES, NB)
    # uniform tiles per block = max over cores (>=1 so PSUM is initialized)
    tiles_b = np.maximum(np.ceil(cnt.max(axis=0) / P).astype(np.int64), 1)  # [NB]
    TT = int(tiles_b.sum())

    tile_block = np.zeros(TT, np.int64)
    seg_off = np.zeros(NB, np.int64)
    t = 0
    for b in range(NB):
        seg_off[b] = t
        n = int(tiles_b[b])
        tile_block[t:t + n] = b
        t += n
    assert t == TT

    # Table rows are PERMUTED: each core's blocks 0..24 (the "A" half,
    # all-gathered mid-layer) land in high rows [24576, 50176); blocks
    # 25..48 ("B", all-gathered at layer end) land in [0, 24576). The
    # signed-idx gather AP starts at 32768 (inside A), so Tile auto-deps
    # gathers on the A collective; B gets explicit deps per window.
    nc_ = np.arange(NP) // SH
    nr_ = np.arange(NP) % SH
    nblk = nr_ // P
    rowmap = np.where(
        nblk < 25,
        24576 + nc_ * 3200 + nr_,
        nc_ * 3072 + (nr_ - 25 * P))
    PAD_IDX = 50175 - 32768  # an A-region row; harmless (S column zero)
    pidx = np.full((NCORES, TT * P), PAD_IDX, np.int64)
    dcol = np.full((NCORES, TT * P), -1.0, np.float32)
    vcol = np.zeros((NCORES, TT * P), np.float32)
    starts = np.concatenate([[0], np.cumsum(cnt.reshape(-1))])
    for c in range(NCORES):
        for b in range(NB):
            k = c * NB + b
            lo, hi = starts[k], starts[k + 1]
            n = hi - lo
            if n == 0:
                continue
            off = seg_off[b] * P
            rows = rowmap[s_src[lo:hi]]
            o2 = np.argsort(-rows, kind="stable")  # A-rows (high) first
            pidx[c, off:off + n] = rows[o2] - 32768
            dcol[c, off:off + n] = ((s_dst[lo:hi] % SH) % P)[o2]
            vcol[c, off:off + n] = inv_deg[s_dst_orig[lo:hi]][o2]

    # Q7 drops trailing-negative idxs per call: the LAST idx of every
    # 1024-idx gather window must be >= 0. Swap a non-negative idx (high
    # src or pad) from the same segment (same dst block -> semantics
    # unchanged) into each bad window's last slot.
    nwin = (TT + WIN - 1) // WIN
    win_last = set((min((w + 1) * WIN, TT) * P) - 1 for w in range(nwin))
    seg_lo = seg_off * P                      # edge offset of each segment
    seg_hi = (seg_off + tiles_b) * P
    for c in range(NCORES):
        for w in range(nwin):
            e1 = min((w + 1) * WIN, TT) * P
            j = e1 - 1
            if pidx[c, j] >= 0:
                continue
            b = int(tile_block[(e1 - 1) // P])
            cand = seg_lo[b] + np.nonzero(pidx[c, seg_lo[b]:seg_hi[b]] >= 0)[0]
            cand = [int(q) for q in cand if int(q) not in win_last]
            if not cand:
                raise ValueError("segment with no high-src edge or pad; "
                                 "unsupported input distribution")
            q = cand[-1]
            for arr in (pidx, dcol, vcol):
                arr[c, q], arr[c, j] = arr[c, j], arr[c, q]

    # classify windows AFTER swaps: window needs the B collective iff any
    # core's window touches a row < 24576 (same flag on all cores - SPMD)
    win_b = np.zeros(nwin, bool)
    for w in range(nwin):
        e0, e1 = w * WIN * P, min((w + 1) * WIN, TT) * P
        win_b[w] = bool((pidx[:, e0:e1] + 32768 < 24576).any())

    idx16 = np.tile(
        pidx.astype(np.int16).reshape(NCORES, TT * P // 16, 16).transpose(0, 2, 1),
        (1, 8, 1))  # [NCORES, 128, TT*8]
    dcol = dcol.reshape(NCORES, TT, P).transpose(0, 2, 1)
    vcol = vcol.reshape(NCORES, TT, P).transpose(0, 2, 1)

    return dict(idx16=idx16, dcol=np.ascontiguousarray(dcol),
                vcol=np.ascontiguousarray(vcol),
                tiles_b=tiles_b, tile_block=tile_block, TT=TT, win_b=win_b,
                newpos=newpos)


def _build_program(meta, use_bin, use_bl, use_g, use_bout):
    import concourse.bacc as bacc
    import concourse.mybir as mybir
    import concourse.tile as tile
    from concourse.tile_rust import add_dep_helper
    from concourse.alu_op_type import AluOpType as ALU

    FP32 = mybir.dt.float32
    I16 = mybir.dt.int16
    AF = mybir.ActivationFunctionType

    TT = meta["TT"]
    tile_block = meta["tile_block"]
    win_b = meta["win_b"]
    K0 = D_IN + (1 if use_bin else 0)

    nc = bacc.Bacc("TRN2", target_bir_lowering=False, debug=False,
                   num_devices=NCORES)

    # ---- I/O ----
    xTo = nc.dram_tensor("xTo", [K0, SH], FP32, kind="ExternalInput")
    Wa = nc.dram_tensor("Wa", [K0, HID], FP32, kind="ExternalInput")
    Wl = nc.dram_tensor("Wl", [N_LAYERS, HID, HID], FP32, kind="ExternalInput")
    Wr = nc.dram_tensor("Wr", [N_LAYERS, HID, HID], FP32, kind="ExternalInput")
    Wout = nc.dram_tensor("Wout", [HID, D_OUT], FP32, kind="ExternalInput")
    iota_d = nc.dram_tensor("iota", [P, P], FP32, kind="ExternalInput")
    ident_d = nc.dram_tensor("ident", [P, P], FP32, kind="ExternalInput")
    idx_d = nc.dram_tensor("idx16", [P, TT * 8], I16, kind="ExternalInput")
    dcol_d = nc.dram_tensor("dcol", [P, TT], FP32, kind="ExternalInput")
    vcol_d = nc.dram_tensor("vcol", [P, TT], FP32, kind="ExternalInput")
    if use_bl:
        blb_d = nc.dram_tensor("blb", [N_LAYERS, P, HID], FP32, kind="ExternalInput")
    if use_g:
        gb_d = nc.dram_tensor("gb", [N_LAYERS, P, HID], FP32, kind="ExternalInput")
        bb_d = nc.dram_tensor("bb", [N_LAYERS, P, HID], FP32, kind="ExternalInput")
    if use_bout:
        bob_d = nc.dram_tensor("bob", [P, D_OUT], FP32, kind="ExternalInput")
    out_d = nc.dram_tensor("out", [SH, D_OUT], FP32, kind="ExternalOutput")

    # ---- internal DRAM ----
    hn0_d = nc.dram_tensor("hn0", [NP, HID], FP32, addr_space="Shared")
    ag_inA = nc.dram_tensor("ag_inA", [25 * P, HID], FP32)
    ag_inB = nc.dram_tensor("ag_inB", [24 * P, HID], FP32)
    ag1_d = nc.dram_tensor("ag1", [NP, HID], FP32, addr_space="Shared")
    ag2_d = nc.dram_tensor("ag2", [NP, HID], FP32, addr_space="Shared")

    with tile.TileContext(nc) as tc:
        with (
            tc.tile_pool(name="const", bufs=1) as cp,
            tc.tile_pool(name="resid", bufs=1) as rp,
            tc.tile_pool(name="work", bufs=4) as wp,
            tc.tile_pool(name="stat", bufs=4) as stp,
            tc.tile_pool(name="pagg", bufs=3, space="PSUM") as pagg,
            tc.tile_pool(name="pmisc", bufs=4, space="PSUM") as pmisc,
        ):
            # ---- constants into SBUF ----
            iota_t = cp.tile([P, P], FP32)
            nc.sync.dma_start(out=iota_t[:], in_=iota_d[:, :])
            ident_t = cp.tile([P, P], FP32)
            nc.sync.dma_start(out=ident_t[:], in_=ident_d[:, :])
            Wa_t = cp.tile([K0, HID], FP32)
            nc.sync.dma_start(out=Wa_t[:], in_=Wa[:, :])
            Wl_t = [cp.tile([HID, HID], FP32, tag=f"wl{i}", name=f"wl{i}") for i in range(3)]
            Wr_t = [cp.tile([HID, HID], FP32, tag=f"wr{i}", name=f"wr{i}") for i in range(3)]
            for i in range(3):
                nc.sync.dma_start(out=Wl_t[i][:], in_=Wl[i, :, :])
                nc.sync.dma_start(out=Wr_t[i][:], in_=Wr[i, :, :])
            Wout_t = cp.tile([HID, D_OUT], FP32)
            nc.sync.dma_start(out=Wout_t[:], in_=Wout[:, :])
            idx_t = cp.tile([P, TT * 8], I16)
            nc.sync.dma_start(out=idx_t[:], in_=idx_d[:, :])
            dcol_t = cp.tile([P, TT], FP32)
            nc.sync.dma_start(out=dcol_t[:], in_=dcol_d[:, :])
            vcol_t = cp.tile([P, TT], FP32)
            nc.sync.dma_start(out=vcol_t[:], in_=vcol_d[:, :])
            if use_bl:
                blb_t = [cp.tile([P, HID], FP32, tag=f"blb{i}", name=f"blb{i}") for i in range(3)]
                for i in range(3):
                    nc.sync.dma_start(out=blb_t[i][:], in_=blb_d[i, :, :])
            if use_g:
                gb_t = [cp.tile([P, HID], FP32, tag=f"gb{i}", name=f"gb{i}") for i in range(3)]
                bb_t = [cp.tile([P, HID], FP32, tag=f"bb{i}", name=f"bb{i}") for i in range(3)]
                for i in range(3):
                    nc.sync.dma_start(out=gb_t[i][:], in_=gb_d[i, :, :])
                    nc.sync.dma_start(out=bb_t[i][:], in_=bb_d[i, :, :])
            if use_bout:
                bob_t = cp.tile([P, D_OUT], FP32)
                nc.sync.dma_start(out=bob_t[:], in_=bob_d[:, :])

            eps_t = cp.tile([P, 1], FP32)
            nc.vector.memset(eps_t[:], LN_EPS)
            h_blk = [rp.tile([P, HID], FP32, tag=f"h{b}", name=f"h{b}") for b in range(NB)]
            hn_blk = [rp.tile([P, HID], FP32, tag=f"hn{b}", name=f"hn{b}") for b in range(NB)]

            def layer_norm_tile(src_ap, dst_ap, li):
                """dst = LN(src) (optionally *g+b). src may be PSUM."""
                st6 = stp.tile([P, 6], FP32, tag="st6")
                nc.vector.bn_stats(st6[:], src_ap)
                mv = stp.tile([P, 2], FP32, tag="mv")
                nc.vector.bn_aggr(mv[:], st6[:])
                sd = stp.tile([P, 1], FP32, tag="sd")
                nc.scalar.activation(sd[:], mv[:, 1:2], AF.Sqrt, bias=eps_t[:])
                rstd = stp.tile([P, 1], FP32, tag="rstd")
                nc.vector.reciprocal(rstd[:], sd[:])
                if use_g:
                    tmp = wp.tile([P, HID], FP32, tag="lnt")
                    nc.vector.tensor_scalar(tmp[:], src_ap, mv[:, 0:1], rstd[:],
                                            ALU.subtract, ALU.mult)
                    nc.vector.tensor_tensor(out=tmp[:], in0=tmp[:],
                                            in1=gb_t[li][:], op=ALU.mult)
                    nc.vector.tensor_tensor(out=dst_ap, in0=tmp[:],
                                            in1=bb_t[li][:], op=ALU.add)
                else:
                    nc.vector.tensor_scalar(dst_ap, src_ap, mv[:, 0:1], rstd[:],
                                            ALU.subtract, ALU.mult)

            # ================= Layer 0 =================
            # own shard only: h0/hn0 resident; hn0 table built by a split
            # AllGather (A mid-pass, B at end) exactly like the other layers.
            agB_i = {}
            with tc.tile_pool(name="l0pool", bufs=1) as l0p:
                xo = l0p.tile([K0, SH], FP32, tag="xo")
                nc.sync.dma_start(out=xo[:], in_=xTo[:, :])
                for b in range(NB):
                    ps = pmisc.tile([P, HID], FP32, tag="pm")
                    nc.tensor.matmul(out=ps[:], lhsT=xo[:, b * P:(b + 1) * P],
                                     rhs=Wa_t[:], start=True, stop=True)
                    nc.scalar.copy(out=h_blk[b][:], in_=ps[:])
                    layer_norm_tile(ps[:], hn_blk[b][:], 0)
                    if b < 25:
                        nc.sync.dma_start(out=ag_inA[b * P:(b + 1) * P, :],
                                          in_=hn_blk[b][:])
                    else:
                        nc.sync.dma_start(out=ag_inB[(b - 25) * P:(b - 24) * P, :],
                                          in_=hn_blk[b][:])
                nc.gpsimd.collective_compute(
                    "AllGather", mybir.AluOpType.bypass,
                    replica_groups=[list(range(NCORES))],
                    ins=[ag_inA[:, :]], outs=[hn0_d[24576:, :]])
                agB_i[0] = nc.gpsimd.collective_compute(
                    "AllGather", mybir.AluOpType.bypass,
                    replica_groups=[list(range(NCORES))],
                    ins=[ag_inB[:, :]], outs=[hn0_d[:24576, :]])

            # ================= Layers 1..3 =================
            nwin = (TT + WIN - 1) // WIN
            lyr_stack = tc.tile_pool(name="gpool", bufs=8)
            gp = lyr_stack.__enter__()
            sp_cm = tc.tile_pool(name="spool", bufs=4); sp = sp_cm.__enter__()
            fp_cm = tc.tile_pool(name="flush", bufs=4); fp = fp_cm.__enter__()
            for li in range(N_LAYERS):
                table = (hn0_d, ag1_d, ag2_d)[li]
                tab_hi = table[32768:, :]  # signed-idx base
                aggT = {}  # block -> sbuf tile
                for w in range(nwin):
                    t0, t1 = w * WIN, min((w + 1) * WIN, TT)
                    wt = t1 - t0
                    g = gp.tile([P, WIN, HID], FP32, tag="g")
                    g_i = nc.gpsimd.dma_gather(
                        g[:, :wt, :], tab_hi,
                        idx_t[:, t0 * 8:t1 * 8], wt * P, wt * P, HID)
                    if win_b[w]:
                        add_dep_helper(g_i.ins, agB_i[li].ins, sync=True,
                                       reason="window reads B rows: wait for AG-B")
                    for t in range(t0, t1):
                        b = int(tile_block[t])
                        first = (t == 0) or (tile_block[t - 1] != b)
                        last = (t == TT - 1) or (tile_block[t + 1] != b)
                        S = sp.tile([P, P], FP32, tag="S")
                        nc.vector.tensor_scalar(
                            S[:], iota_t[:], dcol_t[:, t:t + 1], vcol_t[:, t:t + 1],
                            ALU.is_equal, ALU.mult)
                        if first:
                            aggT[b] = pagg.tile([P, P], FP32, tag="paggT", name=f"paggT{b}")
                        nc.tensor.matmul(
                            out=aggT[b][:],
                            lhsT=g[:, t - t0, :],
                            rhs=S[:], start=first, stop=last)
                        if last:
                            # flush block b: dense + residual + relu (+ LN)
                            aggT_s = fp.tile([P, P], FP32, tag="aggTs")
                            nc.scalar.copy(out=aggT_s[:], in_=aggT[b][:])
                            ph = pmisc.tile([P, P], FP32, tag="pm")
                            nc.tensor.transpose(out=ph[:], in_=hn_blk[b][:],
                                                identity=ident_t[:])
                            hnT_s = fp.tile([P, P], FP32, tag="hnTs")
                            nc.scalar.copy(out=hnT_s[:], in_=ph[:])
                            pc = pmisc.tile([P, HID], FP32, tag="pm")
                            nc.tensor.matmul(out=pc[:], lhsT=aggT_s[:],
                                             rhs=Wl_t[li][:], start=True, stop=False)
                            nc.tensor.matmul(out=pc[:], lhsT=hnT_s[:],
                                             rhs=Wr_t[li][:], start=False, stop=True)
                            hin = wp.tile([P, HID], FP32, tag="hin")
                            nc.vector.tensor_tensor(out=hin[:], in0=h_blk[b][:],
                                                    in1=pc[:], op=ALU.add)
                            if use_bl:
                                nc.vector.tensor_tensor(out=hin[:], in0=hin[:],
                                                        in1=blb_t[li][:], op=ALU.add)
                            nc.vector.tensor_relu(out=h_blk[b][:], in_=hin[:])
                            if li < N_LAYERS - 1:
                                layer_norm_tile(h_blk[b][:], hn_blk[b][:], li + 1)
                                if b < 25:
                                    nc.sync.dma_start(
                                        out=ag_inA[b * P:(b + 1) * P, :],
                                        in_=hn_blk[b][:])
                                else:
                                    nc.sync.dma_start(
                                        out=ag_inB[(b - 25) * P:(b - 24) * P, :],
                                        in_=hn_blk[b][:])
                            else:
                                ph3 = pmisc.tile([P, P], FP32, tag="pm")
                                nc.tensor.transpose(out=ph3[:], in_=h_blk[b][:],
                                                    identity=ident_t[:])
                                h3T_s = fp.tile([P, P], FP32, tag="h3Ts")
                                nc.scalar.copy(out=h3T_s[:], in_=ph3[:])
                                po = pmisc.tile([P, D_OUT], FP32, tag="pm")
                                nc.tensor.matmul(out=po[:], lhsT=h3T_s[:],
                                                 rhs=Wout_t[:], start=True, stop=True)
                                o_s = wp.tile([P, D_OUT], FP32, tag="outs")
                                if use_bout:
                                    nc.vector.tensor_tensor(out=o_s[:], in0=po[:],
                                                            in1=bob_t[:], op=ALU.add)
                                else:
                                    nc.scalar.copy(out=o_s[:], in_=po[:])
                                nc.sync.dma_start(out=out_d[b * P:(b + 1) * P, :],
                                                  in_=o_s[:])
                if li < N_LAYERS - 1:
                    ag_out = (ag1_d, ag2_d)[li]
                    nc.gpsimd.collective_compute(
                        "AllGather", mybir.AluOpType.bypass,
                        replica_groups=[list(range(NCORES))],
                        ins=[ag_inA[:, :]], outs=[ag_out[24576:, :]])
                    agB_i[li + 1] = nc.gpsimd.collective_compute(
                        "AllGather", mybir.AluOpType.bypass,
                        replica_groups=[list(range(NCORES))],
                        ins=[ag_inB[:, :]], outs=[ag_out[:24576, :]])
            fp_cm.__exit__(None, None, None)
            sp_cm.__exit__(None, None, None)
            lyr_stack.__exit__(None, None, None)

    nc.compile()
    return nc


def _get_runner(inputs):
    key = (hash(np.asarray(inputs["edge_index"]).tobytes()),
           tuple(np.asarray(inputs["x"]).shape))
    if key in _CACHE:
        return _CACHE[key]

    meta = _host_prep(inputs["edge_index"])
    use_bin = bool(np.any(np.asarray(inputs["b_in"]) != 0))
    use_bl = bool(np.any(np.asarray(inputs["bl"]) != 0))
    use_g = bool(np.any(np.asarray(inputs["ln_g"]) != 1.0)
                 or np.any(np.asarray(inputs["ln_b"]) != 0))
    use_bout = bool(np.any(np.asarray(inputs["b_out"]) != 0))
    nc = _build_program(meta, use_bin, use_bl, use_g, use_bout)
    from runner_embedded import SpmdRunner
    runner = SpmdRunner(nc, NCORES)
    _CACHE[key] = (runner, meta, use_bin, use_bl, use_g, use_bout)
    return _CACHE[key]


def kernel(**inputs):
    runner, meta, use_bin, use_bl, use_g, use_bout = _get_runner(inputs)

    x = np.asarray(inputs["x"], np.float32)
    W_in = np.asarray(inputs["W_in"], np.float32)
    b_in = np.asarray(inputs["b_in"], np.float32)
    Wl = np.asarray(inputs["Wl"], np.float32)
    bl = np.asarray(inputs["bl"], np.float32)
    Wr = np.asarray(inputs["Wr"], np.float32)
    ln_g = np.asarray(inputs["ln_g"], np.float32)
    ln_b = np.asarray(inputs["ln_b"], np.float32)
    W_out = np.asarray(inputs["W_out"], np.float32)
    b_out = np.asarray(inputs["b_out"], np.float32)

    x_pad = np.zeros((NP, D_IN), np.float32)
    x_pad[meta["newpos"][:N_NODES]] = x[:N_NODES] if len(x) >= N_NODES else x
    x_pad[meta["newpos"][N_NODES:]] = 0.0
    xT = x_pad.T  # [16, NP] in permuted slot order
    if use_bin:
        xTa = np.concatenate([xT, np.ones((1, NP), np.float32)], axis=0)
        Wa = np.concatenate([W_in, b_in[None, :]], axis=0)
    else:
        xTa, Wa = xT, W_in
    iota = np.tile(np.arange(P, dtype=np.float32)[None, :], (P, 1))
    ident = np.eye(P, dtype=np.float32)

    base = {
        "Wa": np.ascontiguousarray(Wa),
        "Wl": Wl, "Wr": Wr, "Wout": W_out,
        "iota": iota, "ident": ident,
    }
    if use_bl:
        base["blb"] = np.tile(bl[:, None, :], (1, P, 1))
    if use_g:
        base["gb"] = np.tile(ln_g[:, None, :], (1, P, 1))
        base["bb"] = np.tile(ln_b[:, None, :], (1, P, 1))
    if use_bout:
        base["bob"] = np.tile(b_out[None, :], (P, 1))

    in_maps = []
    for c in range(NCORES):
        m = dict(base)
        m["xTo"] = np.ascontiguousarray(xTa[:, c * SH:(c + 1) * SH])
        m["idx16"] = meta["idx16"][c]
        m["dcol"] = meta["dcol"][c]
        m["vcol"] = meta["vcol"][c]
        in_maps.append(m)

    runner.stage(in_maps)
    res = runner.results()
    out_new = np.concatenate([res[c]["out"] for c in range(NCORES)], axis=0)
    return out_new[meta["newpos"][:N_NODES]].astype(np.float32)


# ---------------------------------------------------------------------------
# embedded PJRT runner (self-contained; mirrors bass2jax.run_bass_via_pjrt)
import types as _types

_runner_mod = _types.ModuleType("runner_embedded")
_runner_src = '''
import sys
sys.path.insert(0, "/opt/trn_rl_repo")
import numpy as np
import jax
from jax.sharding import Mesh, PartitionSpec, NamedSharding
from jax.experimental.shard_map import shard_map
import concourse.mybir as mybir
from concourse.bass2jax import _bass_exec_p, install_neuronx_cc_hook, partition_id_tensor


class SpmdRunner:
    def __init__(self, nc, n_cores=8):
        install_neuronx_cc_hook()
        self.nc = nc
        self.n_cores = n_cores
        partition_name = nc.partition_id_tensor.name if nc.partition_id_tensor else None
        in_names, out_names, out_avals, zero_outs = [], [], [], []
        for alloc in nc.m.functions[0].allocations:
            if not isinstance(alloc, mybir.MemoryLocationSet):
                continue
            name = alloc.memorylocations[0].name
            if alloc.kind == "ExternalInput":
                if name != partition_name and name != (nc.dbg_addr.name if nc.dbg_addr else None):
                    in_names.append(name)
            elif alloc.kind == "ExternalOutput":
                shape = tuple(alloc.tensor_shape)
                dtype = mybir.dt.np(alloc.dtype)
                out_names.append(name)
                out_avals.append(jax.core.ShapedArray(shape, dtype))
                zero_outs.append(np.zeros(shape, dtype))
        self.in_names, self.out_names = in_names, out_names
        self.out_avals, self.zero_outs = out_avals, zero_outs
        n_params, n_outs = len(in_names), len(out_names)
        self.n_params = n_params
        all_names = list(in_names) + list(out_names)
        if nc.dbg_addr is not None:
            all_names.append(nc.dbg_addr.name)
        if partition_name is not None:
            all_names.append(partition_name)
        has_dbg = nc.dbg_addr is not None

        def _body(*args):
            operands = list(args)
            if has_dbg:
                operands.append(np.zeros((1, 2), np.uint32))
            if partition_name is not None:
                operands.append(partition_id_tensor())
            outs = _bass_exec_p.bind(
                *operands,
                out_avals=tuple(out_avals),
                in_names=tuple(all_names),
                out_names=tuple(out_names),
                lowering_input_output_aliases=(),
                sim_require_finite=True,
                sim_require_nnan=True,
                nc=nc,
            )
            return tuple(outs)

        devices = jax.devices()[:n_cores]
        self.mesh = Mesh(np.asarray(devices), ("core",))
        self.sharding = NamedSharding(self.mesh, PartitionSpec("core"))
        in_specs = (PartitionSpec("core"),) * (n_params + n_outs)
        out_specs = (PartitionSpec("core"),) * n_outs
        self.fn = jax.jit(
            shard_map(_body, mesh=self.mesh, in_specs=in_specs,
                      out_specs=out_specs, check_rep=False),
            keep_unused=True,
        )
        self.dev_in = None

    def stage(self, in_maps):
        per_core = [[np.asarray(m[n]) for n in self.in_names] for m in in_maps]
        concat_in = [
            np.concatenate([per_core[c][i] for c in range(self.n_cores)], axis=0)
            for i in range(self.n_params)
        ]
        concat_zero = [
            np.zeros((self.n_cores * z.shape[0], *z.shape[1:]), z.dtype)
            for z in self.zero_outs
        ]
        self.dev_in = [jax.device_put(a, self.sharding) for a in concat_in + concat_zero]
        return self

    def run(self):
        outs = self.fn(*self.dev_in)
        jax.block_until_ready(outs)
        return outs

    def results(self):
        outs = self.run()
        return [
            {name: np.asarray(outs[i]).reshape(self.n_cores, *self.out_avals[i].shape)[c]
             for i, name in enumerate(self.out_names)}
            for c in range(self.n_cores)
        ]
'''
exec(compile(_runner_src, "runner_embedded", "exec"), _runner_mod.__dict__)
sys.modules["runner_embedded"] = _runner_mod


# revision 20
# speedup vs baseline: 2.7032x; 1.7295x over previous
"""DropEdge GraphSAGE (eval mode) on 8 Trainium2 NeuronCores.

Strategy (graph/data parallel, per sharding hint):
- Nodes padded 50000->50176 = 8 cores x 6272 (49 blocks of 128).
- Edges sharded by destination core; per core grouped by (dst block, src
  parity), sorted by src, padded to 128-edge tiles (uniform tile counts
  across cores so the SPMD program is identical).
- Layer 0 (x @ W_in + LN): computed fully on every core (replicated) to
  avoid an AllGather; own-shard h/hn also kept in SBUF.
- Aggregation per layer: indirect DMA gather of *row pairs* (1024B
  descriptors, idx = src>>1 fits int16) from the replicated hn table in
  DRAM; one-hot selection matrices S (built on DVE from iota vs dst ids,
  with 1/deg folded in) turn segment-sum into PE matmuls accumulating in
  PSUM per dst block, feature-major: aggT = G^T S.
- Dense part: conv = aggT^T Wl + hnT^T Wr in PSUM; residual+relu; LN via
  bn_stats/bn_aggr; AllGather of the new hn shard between layers.
- Final layer: h3^T W_out -> per-core [6272,4]; host concatenates + trims.
"""
import sys
sys.path.insert(0, "/opt/trn_rl_repo")
import numpy as np

N_NODES = 50000
N_EDGES = 800000
D_IN = 16
HID = 128
D_OUT = 4
N_LAYERS = 3
LN_EPS = 1e-5

P = 128
NCORES = 8
NP = 50176            # padded nodes
SH = NP // NCORES     # 6272 per core
NB = SH // P          # 49 blocks per core
NT_FULL = NP // P     # 392 node tiles
WIN = 8               # tiles per gather window (<=1024 idxs)

_CACHE = {}


def _host_prep(edge_index):
    src = np.asarray(edge_index[0], dtype=np.int64)
    dst = np.asarray(edge_index[1], dtype=np.int64)
    deg = np.bincount(dst, minlength=NP).astype(np.float32)
    inv_deg = 1.0 / np.maximum(deg, 1.0)

    # Per-core balanced repacking: each core assigns its own 6272 nodes to
    # its 49 blocks so block in-degree sums are flat (~2041 < 2048 = 16
    # tiles); blocks then sorted by sum desc so the shared per-local-index
    # tile counts are minimal. newpos[n] = new global slot of node n.
    import heapq
    CAP = 2047          # 16-tile capacity with margin
    REG_TARGET = 2040   # regular-block mean target
    newpos = np.empty(NP, np.int64)
    for c in range(NCORES):
        nodes_c = np.arange(c * SH, min((c + 1) * SH, NP))
        degs_c = deg[nodes_c]
        o_ = list(np.argsort(-degs_c, kind="stable"))
        total_c = float(degs_c.sum())
        # spill block: seed with highest-degree nodes until the rest fits
        # 48 regular blocks at <= REG_TARGET mean; pad with lowest-degree.
        spill_need = max(total_c - 48 * REG_TARGET, 0.0)
        blk0, s0 = [], 0.0
        while s0 < spill_need and len(blk0) < P and o_:
            i_ = o_.pop(0)
            blk0.append(nodes_c[i_]); s0 += degs_c[i_]
        while len(blk0) < P and o_:
            i_ = o_.pop()            # lowest degree from the tail
            blk0.append(nodes_c[i_]); s0 += degs_c[i_]
        # balance the rest over 48 blocks, capped at CAP
        heap = [(0.0, 0, b_) for b_ in range(48)]
        heapq.heapify(heap)
        blocks = [[] for _ in range(48)]
        sums = np.zeros(48)
        for i_ in o_:
            popped = []
            placed = False
            while heap:
                s_, _, b_ = heapq.heappop(heap)
                if len(blocks[b_]) >= P:
                    continue
                if s_ + degs_c[i_] <= CAP or not placed:
                    if s_ + degs_c[i_] <= CAP:
                        blocks[b_].append(nodes_c[i_])
                        sums[b_] += degs_c[i_]
                        heapq.heappush(heap, (sums[b_], len(blocks[b_]), b_))
                        placed = True
                        break
                popped.append((s_, _, b_))
            for e_ in popped:
                heapq.heappush(heap, e_)
            if not placed:
                # forced: lowest-sum block with space
                b_ = min((b for b in range(48) if len(blocks[b]) < P),
                         key=lambda b: sums[b])
                blocks[b_].append(nodes_c[i_]); sums[b_] += degs_c[i_]
                heapq.heappush(heap, (sums[b_], len(blocks[b_]), b_))
        all_blocks = [blk0] + blocks
        all_sums = np.concatenate([[s0], sums])
        border = np.argsort(-all_sums, kind="stable")
        for bl, g in enumerate(border):
            for sl, n_ in enumerate(all_blocks[g]):
                newpos[n_] = c * SH + bl * P + sl

    src_n = newpos[src]
    dst_n = newpos[dst]
    core = dst_n // SH
    block = (dst_n % SH) // P
    order = np.lexsort((src_n, block, core))
    s_src, s_dst, s_core, s_blk = src_n[order], dst_n[order], core[order], block[order]
    s_dst_orig = dst[order]  # for inv_deg lookup (deg is per original node)
    key = s_core * NB + s_blk
    cnt = np.bincount(key, minlength=NCORES * NB).reshape(NCORES, NB)
    # uniform tiles per block = max over cores (>=1 so PSUM is initialized)
    tiles_b = np.maximum(np.ceil(cnt.max(axis=0) / P).astype(np.int64), 1)  # [NB]
    TT = int(tiles_b.sum())

    tile_block = np.zeros(TT, np.int64)
    seg_off = np.zeros(NB, np.int64)
    t = 0
    for b in range(NB):
        seg_off[b] = t
        n = int(tiles_b[b])
        tile_block[t:t + n] = b
        t += n
    assert t == TT

    # Table rows are PERMUTED: each core's blocks 0..24 (the "A" half,
    # all-gathered mid-layer) land in high rows [24576, 50176); blocks
    # 25..48 ("B", all-gathered at layer end) land in [0, 24576). The
    # signed-idx gather AP starts at 32768 (inside A), so Tile auto-deps
    # gathers on the A collective; B gets explicit deps per window.
    nc_ = np.arange(NP) // SH
    nr_ = np.arange(NP) % SH
    nblk = nr_ // P
    rowmap = np.where(
        nblk < 36,
        13312 + nc_ * 4608 + nr_,
        nc_ * 1664 + (nr_ - 36 * P))
    PAD_IDX = 50175 - 32768  # an A-region row; harmless (S column zero)
    pidx = np.full((NCORES, TT * P), PAD_IDX, np.int64)
    dcol = np.full((NCORES, TT * P), -1.0, np.float32)
    vcol = np.zeros((NCORES, TT * P), np.float32)
    starts = np.concatenate([[0], np.cumsum(cnt.reshape(-1))])
    for c in range(NCORES):
        for b in range(NB):
            k = c * NB + b
            lo, hi = starts[k], starts[k + 1]
            n = hi - lo
            if n == 0:
                continue
            off = seg_off[b] * P
            rows = rowmap[s_src[lo:hi]]
            o2 = np.argsort(-rows, kind="stable")  # A-rows (high) first
            pidx[c, off:off + n] = rows[o2] - 32768
            dcol[c, off:off + n] = ((s_dst[lo:hi] % SH) % P)[o2]
            vcol[c, off:off + n] = inv_deg[s_dst_orig[lo:hi]][o2]

    # Q7 drops trailing-negative idxs per call: the LAST idx of every
    # 1024-idx gather window must be >= 0. Swap a non-negative idx (high
    # src or pad) from the same segment (same dst block -> semantics
    # unchanged) into each bad window's last slot.
    nwin = (TT + WIN - 1) // WIN
    win_last = set((min((w + 1) * WIN, TT) * P) - 1 for w in range(nwin))
    seg_lo = seg_off * P                      # edge offset of each segment
    seg_hi = (seg_off + tiles_b) * P
    for c in range(NCORES):
        for w in range(nwin):
            e1 = min((w + 1) * WIN, TT) * P
            j = e1 - 1
            if pidx[c, j] >= 0:
                continue
            b = int(tile_block[(e1 - 1) // P])
            cand = seg_lo[b] + np.nonzero(pidx[c, seg_lo[b]:seg_hi[b]] >= 0)[0]
            cand = [int(q) for q in cand if int(q) not in win_last]
            if not cand:
                raise ValueError("segment with no high-src edge or pad; "
                                 "unsupported input distribution")
            q = cand[-1]
            for arr in (pidx, dcol, vcol):
                arr[c, q], arr[c, j] = arr[c, j], arr[c, q]

    # classify windows AFTER swaps: window needs the B collective iff any
    # core's window touches a row < 24576 (same flag on all cores - SPMD)
    win_b = np.zeros(nwin, bool)
    for w in range(nwin):
        e0, e1 = w * WIN * P, min((w + 1) * WIN, TT) * P
        win_b[w] = bool((pidx[:, e0:e1] + 32768 < 13312).any())

    idx16 = np.tile(
        pidx.astype(np.int16).reshape(NCORES, TT * P // 16, 16).transpose(0, 2, 1),
        (1, 8, 1))  # [NCORES, 128, TT*8]
    dcol = dcol.reshape(NCORES, TT, P).transpose(0, 2, 1)
    vcol = vcol.reshape(NCORES, TT, P).transpose(0, 2, 1)

    return dict(idx16=idx16, dcol=np.ascontiguousarray(dcol),
                vcol=np.ascontiguousarray(vcol),
                tiles_b=tiles_b, tile_block=tile_block, TT=TT, win_b=win_b,
                newpos=newpos)


def _build_program(meta, use_bin, use_bl, use_g, use_bout):
    import concourse.bacc as bacc
    import concourse.mybir as mybir
    import concourse.tile as tile
    from concourse.tile_rust import add_dep_helper
    from concourse.alu_op_type import AluOpType as ALU

    FP32 = mybir.dt.float32
    I16 = mybir.dt.int16
    AF = mybir.ActivationFunctionType

    TT = meta["TT"]
    tile_block = meta["tile_block"]
    win_b = meta["win_b"]
    K0 = D_IN + (1 if use_bin else 0)

    nc = bacc.Bacc("TRN2", target_bir_lowering=False, debug=False,
                   num_devices=NCORES)

    # ---- I/O ----
    xTo = nc.dram_tensor("xTo", [K0, SH], FP32, kind="ExternalInput")
    Wa = nc.dram_tensor("Wa", [K0, HID], FP32, kind="ExternalInput")
    Wl = nc.dram_tensor("Wl", [N_LAYERS, HID, HID], FP32, kind="ExternalInput")
    Wr = nc.dram_tensor("Wr", [N_LAYERS, HID, HID], FP32, kind="ExternalInput")
    Wout = nc.dram_tensor("Wout", [HID, D_OUT], FP32, kind="ExternalInput")
    iota_d = nc.dram_tensor("iota", [P, P], FP32, kind="ExternalInput")
    ident_d = nc.dram_tensor("ident", [P, P], FP32, kind="ExternalInput")
    idx_d = nc.dram_tensor("idx16", [P, TT * 8], I16, kind="ExternalInput")
    dcol_d = nc.dram_tensor("dcol", [P, TT], FP32, kind="ExternalInput")
    vcol_d = nc.dram_tensor("vcol", [P, TT], FP32, kind="ExternalInput")
    if use_bl:
        blb_d = nc.dram_tensor("blb", [N_LAYERS, P, HID], FP32, kind="ExternalInput")
    if use_g:
        gb_d = nc.dram_tensor("gb", [N_LAYERS, P, HID], FP32, kind="ExternalInput")
        bb_d = nc.dram_tensor("bb", [N_LAYERS, P, HID], FP32, kind="ExternalInput")
    if use_bout:
        bob_d = nc.dram_tensor("bob", [P, D_OUT], FP32, kind="ExternalInput")
    out_d = nc.dram_tensor("out", [SH, D_OUT], FP32, kind="ExternalOutput")

    # ---- internal DRAM ----
    hn0_d = nc.dram_tensor("hn0", [NP, HID], FP32, addr_space="Shared")
    ag_inA = nc.dram_tensor("ag_inA", [36 * P, HID], FP32)
    ag_inB = nc.dram_tensor("ag_inB", [13 * P, HID], FP32)
    ag1_d = nc.dram_tensor("ag1", [NP, HID], FP32, addr_space="Shared")
    ag2_d = nc.dram_tensor("ag2", [NP, HID], FP32, addr_space="Shared")

    with tile.TileContext(nc) as tc:
        with (
            tc.tile_pool(name="const", bufs=1) as cp,
            tc.tile_pool(name="resid", bufs=1) as rp,
            tc.tile_pool(name="work", bufs=4) as wp,
            tc.tile_pool(name="stat", bufs=4) as stp,
            tc.tile_pool(name="pagg", bufs=3, space="PSUM") as pagg,
            tc.tile_pool(name="pmisc", bufs=4, space="PSUM") as pmisc,
        ):
            # ---- constants into SBUF ----
            iota_t = cp.tile([P, P], FP32)
            nc.sync.dma_start(out=iota_t[:], in_=iota_d[:, :])
            ident_t = cp.tile([P, P], FP32)
            nc.sync.dma_start(out=ident_t[:], in_=ident_d[:, :])
            Wa_t = cp.tile([K0, HID], FP32)
            nc.sync.dma_start(out=Wa_t[:], in_=Wa[:, :])
            Wl_t = [cp.tile([HID, HID], FP32, tag=f"wl{i}", name=f"wl{i}") for i in range(3)]
            Wr_t = [cp.tile([HID, HID], FP32, tag=f"wr{i}", name=f"wr{i}") for i in range(3)]
            for i in range(3):
                nc.sync.dma_start(out=Wl_t[i][:], in_=Wl[i, :, :])
                nc.sync.dma_start(out=Wr_t[i][:], in_=Wr[i, :, :])
            Wout_t = cp.tile([HID, D_OUT], FP32)
            nc.sync.dma_start(out=Wout_t[:], in_=Wout[:, :])
            idx_t = cp.tile([P, TT * 8], I16)
            nc.sync.dma_start(out=idx_t[:], in_=idx_d[:, :])
            dcol_t = cp.tile([P, TT], FP32)
            nc.sync.dma_start(out=dcol_t[:], in_=dcol_d[:, :])
            vcol_t = cp.tile([P, TT], FP32)
            nc.sync.dma_start(out=vcol_t[:], in_=vcol_d[:, :])
            if use_bl:
                blb_t = [cp.tile([P, HID], FP32, tag=f"blb{i}", name=f"blb{i}") for i in range(3)]
                for i in range(3):
                    nc.sync.dma_start(out=blb_t[i][:], in_=blb_d[i, :, :])
            if use_g:
                gb_t = [cp.tile([P, HID], FP32, tag=f"gb{i}", name=f"gb{i}") for i in range(3)]
                bb_t = [cp.tile([P, HID], FP32, tag=f"bb{i}", name=f"bb{i}") for i in range(3)]
                for i in range(3):
                    nc.sync.dma_start(out=gb_t[i][:], in_=gb_d[i, :, :])
                    nc.sync.dma_start(out=bb_t[i][:], in_=bb_d[i, :, :])
            if use_bout:
                bob_t = cp.tile([P, D_OUT], FP32)
                nc.sync.dma_start(out=bob_t[:], in_=bob_d[:, :])

            eps_t = cp.tile([P, 1], FP32)
            nc.vector.memset(eps_t[:], LN_EPS)
            h_blk = [rp.tile([P, HID], FP32, tag=f"h{b}", name=f"h{b}") for b in range(NB)]
            hn_blk = [rp.tile([P, HID], FP32, tag=f"hn{b}", name=f"hn{b}") for b in range(NB)]

            def layer_norm_tile(src_ap, dst_ap, li):
                """dst = LN(src) (optionally *g+b). src may be PSUM."""
                st6 = stp.tile([P, 6], FP32, tag="st6")
                nc.vector.bn_stats(st6[:], src_ap)
                mv = stp.tile([P, 2], FP32, tag="mv")
                nc.vector.bn_aggr(mv[:], st6[:])
                sd = stp.tile([P, 1], FP32, tag="sd")
                nc.scalar.activation(sd[:], mv[:, 1:2], AF.Sqrt, bias=eps_t[:])
                rstd = stp.tile([P, 1], FP32, tag="rstd")
                nc.vector.reciprocal(rstd[:], sd[:])
                if use_g:
                    tmp = wp.tile([P, HID], FP32, tag="lnt")
                    nc.vector.tensor_scalar(tmp[:], src_ap, mv[:, 0:1], rstd[:],
                                            ALU.subtract, ALU.mult)
                    nc.vector.tensor_tensor(out=tmp[:], in0=tmp[:],
                                            in1=gb_t[li][:], op=ALU.mult)
                    nc.vector.tensor_tensor(out=dst_ap, in0=tmp[:],
                                            in1=bb_t[li][:], op=ALU.add)
                else:
                    nc.vector.tensor_scalar(dst_ap, src_ap, mv[:, 0:1], rstd[:],
                                            ALU.subtract, ALU.mult)

            # ================= Layer 0 =================
            # own shard only: h0/hn0 resident; hn0 table built by a split
            # AllGather (A mid-pass, B at end) exactly like the other layers.
            agB_i = {}
            with tc.tile_pool(name="l0pool", bufs=1) as l0p:
                xo = l0p.tile([K0, SH], FP32, tag="xo")
                nc.sync.dma_start(out=xo[:], in_=xTo[:, :])
                for b in range(NB):
                    ps = pmisc.tile([P, HID], FP32, tag="pm")
                    nc.tensor.matmul(out=ps[:], lhsT=xo[:, b * P:(b + 1) * P],
                                     rhs=Wa_t[:], start=True, stop=True)
                    nc.scalar.copy(out=h_blk[b][:], in_=ps[:])
                    layer_norm_tile(ps[:], hn_blk[b][:], 0)
                    if b < 36:
                        nc.sync.dma_start(out=ag_inA[b * P:(b + 1) * P, :],
                                          in_=hn_blk[b][:])
                    else:
                        nc.sync.dma_start(out=ag_inB[(b - 36) * P:(b - 35) * P, :],
                                          in_=hn_blk[b][:])
                nc.gpsimd.collective_compute(
                    "AllGather", mybir.AluOpType.bypass,
                    replica_groups=[list(range(NCORES))],
                    ins=[ag_inA[:, :]], outs=[hn0_d[13312:, :]])
                agB_i[0] = nc.gpsimd.collective_compute(
                    "AllGather", mybir.AluOpType.bypass,
                    replica_groups=[list(range(NCORES))],
                    ins=[ag_inB[:, :]], outs=[hn0_d[:13312, :]])

            # ================= Layers 1..3 =================
            nwin = (TT + WIN - 1) // WIN
            lyr_stack = tc.tile_pool(name="gpool", bufs=8)
            gp = lyr_stack.__enter__()
            sp_cm = tc.tile_pool(name="spool", bufs=4); sp = sp_cm.__enter__()
            fp_cm = tc.tile_pool(name="flush", bufs=4); fp = fp_cm.__enter__()
            for li in range(N_LAYERS):
                table = (hn0_d, ag1_d, ag2_d)[li]
                tab_hi = table[32768:, :]  # signed-idx base
                aggT = {}  # block -> sbuf tile
                for w in range(nwin):
                    t0, t1 = w * WIN, min((w + 1) * WIN, TT)
                    wt = t1 - t0
                    g = gp.tile([P, WIN, HID], FP32, tag="g")
                    g_i = nc.gpsimd.dma_gather(
                        g[:, :wt, :], tab_hi,
                        idx_t[:, t0 * 8:t1 * 8], wt * P, wt * P, HID)
                    if win_b[w]:
                        add_dep_helper(g_i.ins, agB_i[li].ins, sync=True,
                                       reason="window reads B rows: wait for AG-B")
                    for t in range(t0, t1):
                        b = int(tile_block[t])
                        first = (t == 0) or (tile_block[t - 1] != b)
                        last = (t == TT - 1) or (tile_block[t + 1] != b)
                        S = sp.tile([P, P], FP32, tag="S")
                        nc.vector.tensor_scalar(
                            S[:], iota_t[:], dcol_t[:, t:t + 1], vcol_t[:, t:t + 1],
                            ALU.is_equal, ALU.mult)
                        if first:
                            aggT[b] = pagg.tile([P, P], FP32, tag="paggT", name=f"paggT{b}")
                        nc.tensor.matmul(
                            out=aggT[b][:],
                            lhsT=g[:, t - t0, :],
                            rhs=S[:], start=first, stop=last)
                        if last:
                            # flush block b: dense + residual + relu (+ LN)
                            aggT_s = fp.tile([P, P], FP32, tag="aggTs")
                            nc.scalar.copy(out=aggT_s[:], in_=aggT[b][:])
                            ph = pmisc.tile([P, P], FP32, tag="pm")
                            nc.tensor.transpose(out=ph[:], in_=hn_blk[b][:],
                                                identity=ident_t[:])
                            hnT_s = fp.tile([P, P], FP32, tag="hnTs")
                            nc.scalar.copy(out=hnT_s[:], in_=ph[:])
                            pc = pmisc.tile([P, HID], FP32, tag="pm")
                            nc.tensor.matmul(out=pc[:], lhsT=aggT_s[:],
                                             rhs=Wl_t[li][:], start=True, stop=False)
                            nc.tensor.matmul(out=pc[:], lhsT=hnT_s[:],
                                             rhs=Wr_t[li][:], start=False, stop=True)
                            hin = wp.tile([P, HID], FP32, tag="hin")
                            nc.vector.tensor_tensor(out=hin[:], in0=h_blk[b][:],
                                                    in1=pc[:], op=ALU.add)
                            if use_bl:
                                nc.vector.tensor_tensor(out=hin[:], in0=hin[:],
                                                        in1=blb_t[li][:], op=ALU.add)
                            nc.vector.tensor_relu(out=h_blk[b][:], in_=hin[:])
                            if li < N_LAYERS - 1:
                                layer_norm_tile(h_blk[b][:], hn_blk[b][:], li + 1)
                                if b < 36:
                                    nc.sync.dma_start(
                                        out=ag_inA[b * P:(b + 1) * P, :],
                                        in_=hn_blk[b][:])
                                else:
                                    nc.sync.dma_start(
                                        out=ag_inB[(b - 36) * P:(b - 35) * P, :],
                                        in_=hn_blk[b][:])
                            else:
                                ph3 = pmisc.tile([P, P], FP32, tag="pm")
                                nc.tensor.transpose(out=ph3[:], in_=h_blk[b][:],
                                                    identity=ident_t[:])
                                h3T_s = fp.tile([P, P], FP32, tag="h3Ts")
                                nc.scalar.copy(out=h3T_s[:], in_=ph3[:])
                                po = pmisc.tile([P, D_OUT], FP32, tag="pm")
                                nc.tensor.matmul(out=po[:], lhsT=h3T_s[:],
                                                 rhs=Wout_t[:], start=True, stop=True)
                                o_s = wp.tile([P, D_OUT], FP32, tag="outs")
                                if use_bout:
                                    nc.vector.tensor_tensor(out=o_s[:], in0=po[:],
                                                            in1=bob_t[:], op=ALU.add)
                                else:
                                    nc.scalar.copy(out=o_s[:], in_=po[:])
                                nc.sync.dma_start(out=out_d[b * P:(b + 1) * P, :],
                                                  in_=o_s[:])
                if li < N_LAYERS - 1:
                    ag_out = (ag1_d, ag2_d)[li]
                    nc.gpsimd.collective_compute(
                        "AllGather", mybir.AluOpType.bypass,
                        replica_groups=[list(range(NCORES))],
                        ins=[ag_inA[:, :]], outs=[ag_out[13312:, :]])
                    agB_i[li + 1] = nc.gpsimd.collective_compute(
                        "AllGather", mybir.AluOpType.bypass,
                        replica_groups=[list(range(NCORES))],
                        ins=[ag_inB[:, :]], outs=[ag_out[:13312, :]])
            fp_cm.__exit__(None, None, None)
            sp_cm.__exit__(None, None, None)
            lyr_stack.__exit__(None, None, None)

    nc.compile()
    return nc


def _get_runner(inputs):
    key = (hash(np.asarray(inputs["edge_index"]).tobytes()),
           tuple(np.asarray(inputs["x"]).shape))
    if key in _CACHE:
        return _CACHE[key]

    meta = _host_prep(inputs["edge_index"])
    use_bin = bool(np.any(np.asarray(inputs["b_in"]) != 0))
    use_bl = bool(np.any(np.asarray(inputs["bl"]) != 0))
    use_g = bool(np.any(np.asarray(inputs["ln_g"]) != 1.0)
                 or np.any(np.asarray(inputs["ln_b"]) != 0))
    use_bout = bool(np.any(np.asarray(inputs["b_out"]) != 0))
    nc = _build_program(meta, use_bin, use_bl, use_g, use_bout)
    from runner_embedded import SpmdRunner
    runner = SpmdRunner(nc, NCORES)
    _CACHE[key] = (runner, meta, use_bin, use_bl, use_g, use_bout)
    return _CACHE[key]


def kernel(**inputs):
    runner, meta, use_bin, use_bl, use_g, use_bout = _get_runner(inputs)

    x = np.asarray(inputs["x"], np.float32)
    W_in = np.asarray(inputs["W_in"], np.float32)
    b_in = np.asarray(inputs["b_in"], np.float32)
    Wl = np.asarray(inputs["Wl"], np.float32)
    bl = np.asarray(inputs["bl"], np.float32)
    Wr = np.asarray(inputs["Wr"], np.float32)
    ln_g = np.asarray(inputs["ln_g"], np.float32)
    ln_b = np.asarray(inputs["ln_b"], np.float32)
    W_out = np.asarray(inputs["W_out"], np.float32)
    b_out = np.asarray(inputs["b_out"], np.float32)

    x_pad = np.zeros((NP, D_IN), np.float32)
    x_pad[meta["newpos"][:N_NODES]] = x[:N_NODES] if len(x) >= N_NODES else x
    x_pad[meta["newpos"][N_NODES:]] = 0.0
    xT = x_pad.T  # [16, NP] in permuted slot order
    if use_bin:
        xTa = np.concatenate([xT, np.ones((1, NP), np.float32)], axis=0)
        Wa = np.concatenate([W_in, b_in[None, :]], axis=0)
    else:
        xTa, Wa = xT, W_in
    iota = np.tile(np.arange(P, dtype=np.float32)[None, :], (P, 1))
    ident = np.eye(P, dtype=np.float32)

    base = {
        "Wa": np.ascontiguousarray(Wa),
        "Wl": Wl, "Wr": Wr, "Wout": W_out,
        "iota": iota, "ident": ident,
    }
    if use_bl:
        base["blb"] = np.tile(bl[:, None, :], (1, P, 1))
    if use_g:
        base["gb"] = np.tile(ln_g[:, None, :], (1, P, 1))
        base["bb"] = np.tile(ln_b[:, None, :], (1, P, 1))
    if use_bout:
        base["bob"] = np.tile(b_out[None, :], (P, 1))

    in_maps = []
    for c in range(NCORES):
        m = dict(base)
        m["xTo"] = np.ascontiguousarray(xTa[:, c * SH:(c + 1) * SH])
        m["idx16"] = meta["idx16"][c]
        m["dcol"] = meta["dcol"][c]
        m["vcol"] = meta["vcol"][c]
        in_maps.append(m)

    runner.stage(in_maps)
    res = runner.results()
    out_new = np.concatenate([res[c]["out"] for c in range(NCORES)], axis=0)
    return out_new[meta["newpos"][:N_NODES]].astype(np.float32)


# ---------------------------------------------------------------------------
# embedded PJRT runner (self-contained; mirrors bass2jax.run_bass_via_pjrt)
import types as _types

_runner_mod = _types.ModuleType("runner_embedded")
_runner_src = '''
import sys
sys.path.insert(0, "/opt/trn_rl_repo")
import numpy as np
import jax
from jax.sharding import Mesh, PartitionSpec, NamedSharding
from jax.experimental.shard_map import shard_map
import concourse.mybir as mybir
from concourse.bass2jax import _bass_exec_p, install_neuronx_cc_hook, partition_id_tensor


class SpmdRunner:
    def __init__(self, nc, n_cores=8):
        install_neuronx_cc_hook()
        self.nc = nc
        self.n_cores = n_cores
        partition_name = nc.partition_id_tensor.name if nc.partition_id_tensor else None
        in_names, out_names, out_avals, zero_outs = [], [], [], []
        for alloc in nc.m.functions[0].allocations:
            if not isinstance(alloc, mybir.MemoryLocationSet):
                continue
            name = alloc.memorylocations[0].name
            if alloc.kind == "ExternalInput":
                if name != partition_name and name != (nc.dbg_addr.name if nc.dbg_addr else None):
                    in_names.append(name)
            elif alloc.kind == "ExternalOutput":
                shape = tuple(alloc.tensor_shape)
                dtype = mybir.dt.np(alloc.dtype)
                out_names.append(name)
                out_avals.append(jax.core.ShapedArray(shape, dtype))
                zero_outs.append(np.zeros(shape, dtype))
        self.in_names, self.out_names = in_names, out_names
        self.out_avals, self.zero_outs = out_avals, zero_outs
        n_params, n_outs = len(in_names), len(out_names)
        self.n_params = n_params
        all_names = list(in_names) + list(out_names)
        if nc.dbg_addr is not None:
            all_names.append(nc.dbg_addr.name)
        if partition_name is not None:
            all_names.append(partition_name)
        has_dbg = nc.dbg_addr is not None

        def _body(*args):
            operands = list(args)
            if has_dbg:
                operands.append(np.zeros((1, 2), np.uint32))
            if partition_name is not None:
                operands.append(partition_id_tensor())
            outs = _bass_exec_p.bind(
                *operands,
                out_avals=tuple(out_avals),
                in_names=tuple(all_names),
                out_names=tuple(out_names),
                lowering_input_output_aliases=(),
                sim_require_finite=True,
                sim_require_nnan=True,
                nc=nc,
            )
            return tuple(outs)

        devices = jax.devices()[:n_cores]
        self.mesh = Mesh(np.asarray(devices), ("core",))
        self.sharding = NamedSharding(self.mesh, PartitionSpec("core"))
        in_specs = (PartitionSpec("core"),) * (n_params + n_outs)
        out_specs = (PartitionSpec("core"),) * n_outs
        self.fn = jax.jit(
            shard_map(_body, mesh=self.mesh, in_specs=in_specs,
                      out_specs=out_specs, check_rep=False),
            keep_unused=True,
        )
        self.dev_in = None

    def stage(self, in_maps):
        per_core = [[np.asarray(m[n]) for n in self.in_names] for m in in_maps]
        concat_in = [
            np.concatenate([per_core[c][i] for c in range(self.n_cores)], axis=0)
            for i in range(self.n_params)
        ]
        concat_zero = [
            np.zeros((self.n_cores * z.shape[0], *z.shape[1:]), z.dtype)
            for z in self.zero_outs
        ]
        self.dev_in = [jax.device_put(a, self.sharding) for a in concat_in + concat_zero]
        return self

    def run(self):
        outs = self.fn(*self.dev_in)
        jax.block_until_ready(outs)
        return outs

    def results(self):
        outs = self.run()
        return [
            {name: np.asarray(outs[i]).reshape(self.n_cores, *self.out_avals[i].shape)[c]
             for i, name in enumerate(self.out_names)}
            for c in range(self.n_cores)
        ]
'''
exec(compile(_runner_src, "runner_embedded", "exec"), _runner_mod.__dict__)
sys.modules["runner_embedded"] = _runner_mod
